# revision 18
# baseline (speedup 1.0000x reference)
"""MultiHeadAttention (RoPE, causal+padding masks, score-averaging with prev)
on 8 Trainium2 NeuronCores.

Sharding: batch*heads across cores — core i handles b = i//4 and heads
[4*(i%4) .. 4*(i%4)+3]. Projection weights are sliced per core (tensor
parallel over heads); output-projection partials are summed on the host
during the gather.

Numerics: fp32 data; matmuls for the projections and scores run as
3-term float32r hi/lo splits (f32r = RNE to 11 mantissa bits, full PE
rate; the 3-term split keeps ~2^-22 accuracy, better than a plain fp32
matmul chain). The attention (w @ v) matmul and the exp-tile transposes
stay fp32.
"""
import os
import sys
import types

import numpy as np

try:
    import concourse.bass as bass
except ImportError:
    sys.path.insert(0, "/opt/trn_rl_repo")
    import concourse.bass as bass

import concourse.mybir as mybir
import concourse.tile as tile
from concourse.bass import ts
from concourse.bass_utils import run_bass_kernel_spmd
from contextlib import ExitStack

F32 = mybir.dt.float32
F32R = mybir.dt.float32r
BF16 = mybir.dt.bfloat16
AF = mybir.ActivationFunctionType
ALU = mybir.AluOpType

N_HEADS = 16
D_MODEL = 1024
B = 2
S = 1024
D_K = 64
HPC = 4
N_CORES = 8
NEG_INF = float("-inf")

LAST_EXEC_TIME_NS = None


# ---------------------------------------------------------------- wait split
def _split_multi_waits(nc, max_waits=1):
    """This walrus build takes at most one semaphore wait per instruction;
    hoist extras onto NoOps just before it on the same engine stream."""
    n = 0
    for f in nc.m.functions:
        for bb in f.blocks:
            insts = bb.instructions
            if not any(
                i.sync_info and i.sync_info.on_wait
                and len(i.sync_info.on_wait) > max_waits
                for i in insts
            ):
                continue
            new = []
            for inst in insts:
                si = inst.sync_info
                if si is not None and si.on_wait and len(si.on_wait) > max_waits:
                    waits = list(si.on_wait)
                    for w in waits[:-max_waits]:
                        n += 1
                        new.append(mybir.InstNoOp(
                            name=f"{inst.name}-wsplit{n}",
                            engine=inst.engine, ins=[], outs=[],
                            sync_info=mybir.SyncInfo(on_wait=[w], on_update=[]),
                        ))
                    inst.sync_info = mybir.SyncInfo(
                        on_wait=waits[-max_waits:], on_update=list(si.on_update))
                new.append(inst)
            bb.instructions = new
    return n


# ---------------------------------------------------------------- ntff hook
def _install_ntff_hook():
    """Wire up the axon NTFF profile hook missing from this image's antenv
    so trace=True works."""
    if "antenv.axon_hooks" in sys.modules:
        return
    try:
        if "/root/.axon_site" not in sys.path:
            sys.path.insert(0, "/root/.axon_site")
        from trn_agent_boot import trn_boot
        hook = trn_boot._ntff_profile_via_ctypes("/opt/axon/libaxon_pjrt.so")
        mod = types.ModuleType("antenv.axon_hooks")
        holder = [hook]
        mod.get_axon_ntff_profile_hook = lambda: holder[0]
        mod.set_axon_ntff_profile_hook = lambda h: holder.__setitem__(0, h)
        sys.modules["antenv.axon_hooks"] = mod
        import antenv
        antenv.axon_hooks = mod
    except Exception:
        pass


# ---------------------------------------------------------------- program
_PROGRAM = None


def _mm3(nc, out_ap, ah, al, bh, bl, start, stop, tile_position=None):
    """3-term f32r hi/lo matmul accumulation: out += a @ b (a=ah+al, ...)."""
    nc.tensor.matmul(out_ap, ah, bh, start=start, stop=False,
                     tile_position=tile_position)
    nc.tensor.matmul(out_ap, ah, bl, start=False, stop=False,
                     tile_position=tile_position)
    nc.tensor.matmul(out_ap, al, bh, start=False, stop=stop,
                     tile_position=tile_position)


def _build_program():
    nc = bass.Bass()

    xqT = nc.dram_tensor("xqT", [D_MODEL, S], F32, kind="ExternalInput")
    xkT = nc.dram_tensor("xkT", [D_MODEL, S], F32, kind="ExternalInput")
    # host-presplit f32r hi/lo weights (values on the f32r grid, fp32 bits)
    wqh = nc.dram_tensor("wqh", [D_MODEL, 256], F32, kind="ExternalInput")
    wql = nc.dram_tensor("wql", [D_MODEL, 256], F32, kind="ExternalInput")
    wkh = nc.dram_tensor("wkh", [D_MODEL, 256], F32, kind="ExternalInput")
    wkl = nc.dram_tensor("wkl", [D_MODEL, 256], F32, kind="ExternalInput")
    wvf = nc.dram_tensor("wvf", [D_MODEL, 256], F32, kind="ExternalInput")
    wph = nc.dram_tensor("wph", [256, D_MODEL], F32, kind="ExternalInput")
    wpl = nc.dram_tensor("wpl", [256, D_MODEL], F32, kind="ExternalInput")
    prevh = nc.dram_tensor("prevh", [HPC, S, S], F32, kind="ExternalInput")
    maskn = nc.dram_tensor("maskn", [S, S], BF16, kind="ExternalInput")
    costab = nc.dram_tensor("costab", [128, S], F32, kind="ExternalInput")
    sintab = nc.dram_tensor("sintab", [128, S], F32, kind="ExternalInput")
    s2 = nc.dram_tensor("s2", [128, D_K], F32, kind="ExternalInput")
    idn = nc.dram_tensor("idn", [128, 128], F32, kind="ExternalInput")

    scr = nc.dram_tensor("scr", [HPC, S, S], F32, kind="ExternalOutput")
    outp = nc.dram_tensor("outp", [D_MODEL, S], F32, kind="ExternalOutput")
    kdbg = bool(os.environ.get("KDBG"))
    if kdbg:
        dbg_e = nc.dram_tensor("dbg_e", [8, 128, S], F32, kind="ExternalOutput")
        dbg_w = nc.dram_tensor("dbg_w", [128, 8 * S], F32, kind="ExternalOutput")
        dbg_at = nc.dram_tensor("dbg_at", [2, 128, S], F32, kind="ExternalOutput")
        dbg_v = nc.dram_tensor("dbg_v", [8, 128, 256], F32, kind="ExternalOutput")

    with tile.TileContext(nc) as tc, ExitStack() as top:
        consts = top.enter_context(tc.tile_pool(name="consts", bufs=1))
        qk_pool = top.enter_context(tc.tile_pool(name="qk", bufs=1))
        v_pool = top.enter_context(tc.tile_pool(name="vp", bufs=1))
        mask_pool = top.enter_context(tc.tile_pool(name="maskp", bufs=1))
        at_pool = top.enter_context(tc.tile_pool(name="atall", bufs=1))

        # rope'd q/k fp32 (partitions: head 2t ch 0-63 | head 2t+1)
        qr = [qk_pool.tile([128, S], F32, tag=f"qr{t}", name=f"qr{t}") for t in range(2)]
        kr = [qk_pool.tile([128, S], F32, tag=f"kr{t}", name=f"kr{t}") for t in range(2)]
        v_t = [v_pool.tile([128, 256], F32, tag=f"v{st}", name=f"v{st}") for st in range(8)]
        ath = [at_pool.tile([128, S], F32R, tag=f"ath{t}", name=f"ath{t}") for t in range(2)]
        atl = [at_pool.tile([128, S], F32R, tag=f"atl{t}", name=f"atl{t}") for t in range(2)]

        def load_rounded(pool, stage_pool, dram, shape, tag, n, engine="gpsimd"):
            """DMA fp32 (already on the f32r grid) then identity-cast to f32r
            tiles — the verifier requires a rounding producer for f32r."""
            tiles = []
            for i in range(n):
                st = stage_pool.tile(shape, F32, tag=f"stg_{tag}", name=f"stg_{tag}")
                nc.sync.dma_start(st[:], dram[ts(i, shape[0]), :])
                t_ = pool.tile(shape, F32R, tag=f"{tag}{i}", name=f"{tag}{i}")
                if engine == "gpsimd":
                    nc.gpsimd.tensor_copy(t_[:], st[:])
                elif engine == "vector":
                    nc.vector.tensor_copy(t_[:], st[:])
                else:
                    nc.scalar.copy(t_[:], st[:])
                tiles.append(t_)
            return tiles

        # ---------------- phase 1+2: loads + q/k projections (f32r 3-term)
        with ExitStack() as ph:
            qraw_pool = ph.enter_context(tc.tile_pool(name="qraw", bufs=1))
            phq = ph.enter_context(ExitStack())
            xin = phq.enter_context(tc.tile_pool(name="xin", bufs=2))
            xsp = phq.enter_context(tc.tile_pool(name="xsp", bufs=2))
            win = phq.enter_context(tc.tile_pool(name="win", bufs=1))
            wstg = phq.enter_context(tc.tile_pool(name="wstg", bufs=3))
            pqk = phq.enter_context(tc.tile_pool(name="pqk", bufs=1, space="PSUM"))

            wq_h, wq_l, wk_h, wk_l = [], [], [], []
            for i in range(8):
                for lst, dram, nm in ((wq_h, wqh, "wqh"), (wq_l, wql, "wql"),
                                      (wk_h, wkh, "wkh"), (wk_l, wkl, "wkl")):
                    st_ = wstg.tile([128, 256], F32, tag=f"stg_{nm}", name=f"stg_{nm}")
                    nc.sync.dma_start(st_[:], dram[ts(i, 128), :])
                    t_ = win.tile([128, 256], F32R, tag=f"{nm}{i}", name=f"{nm}{i}")
                    nc.vector.tensor_copy(t_[:], st_[:])
                    lst.append(t_)

            pq = [pqk.tile([128, 512], F32, tag=f"pq{t}{c}", name=f"pq{t}{c}")
                  for t in range(2) for c in range(2)]
            pk = [pqk.tile([128, 512], F32, tag=f"pk{t}{c}", name=f"pk{t}{c}")
                  for t in range(2) for c in range(2)]
            for i in range(8):
                xq_f = xin.tile([128, S], F32, tag="xqf", name="xqf")
                nc.sync.dma_start(xq_f[:], xqT[ts(i, 128), :])
                xk_f = xin.tile([128, S], F32, tag="xkf", name="xkf")
                nc.sync.dma_start(xk_f[:], xkT[ts(i, 128), :])
                xq_h = xsp.tile([128, S], F32R, tag="xqh", name="xqh")
                xq_l = xsp.tile([128, S], F32R, tag="xql", name="xql")
                nc.scalar.copy(xq_h[:], xq_f[:])
                nc.vector.tensor_tensor(xq_l[:], xq_f[:], xq_h[:].bitcast(F32),
                                        ALU.subtract)
                xk_h = xsp.tile([128, S], F32R, tag="xkh", name="xkh")
                xk_l = xsp.tile([128, S], F32R, tag="xkl", name="xkl")
                nc.scalar.copy(xk_h[:], xk_f[:])
                nc.vector.tensor_tensor(xk_l[:], xk_f[:], xk_h[:].bitcast(F32),
                                        ALU.subtract)
                for t in range(2):
                    for c in range(2):
                        _mm3(nc, pq[2 * t + c][:],
                             wq_h[i][:, ts(t, 128)], wq_l[i][:, ts(t, 128)],
                             xq_h[:, ts(c, 512)], xq_l[:, ts(c, 512)],
                             start=(i == 0), stop=(i == 7))
                        _mm3(nc, pk[2 * t + c][:],
                             wk_h[i][:, ts(t, 128)], wk_l[i][:, ts(t, 128)],
                             xk_h[:, ts(c, 512)], xk_l[:, ts(c, 512)],
                             start=(i == 0), stop=(i == 7))

            qraw = [qraw_pool.tile([128, S], F32, tag=f"qq{t}", name=f"qq{t}") for t in range(2)]
            kraw = [qraw_pool.tile([128, S], F32, tag=f"kk{t}", name=f"kk{t}") for t in range(2)]
            for t in range(2):
                for c in range(2):
                    nc.scalar.copy(qraw[t][:, ts(c, 512)], pq[2 * t + c][:])
                    nc.scalar.copy(kraw[t][:, ts(c, 512)], pk[2 * t + c][:])
            phq.close()  # release the q/k psum banks before the v/rope pools

            # ---------------- v projection + rope (same scope so qraw lives)
            with ExitStack() as ph2:
                xin2 = ph2.enter_context(tc.tile_pool(name="xin2", bufs=2))
                xsp2 = ph2.enter_context(tc.tile_pool(name="xsp2", bufs=2))
                win2 = ph2.enter_context(tc.tile_pool(name="win2", bufs=1))
                wstg2 = ph2.enter_context(tc.tile_pool(name="wstg2", bufs=3))
                rconsts = ph2.enter_context(tc.tile_pool(name="rconsts", bufs=1))

                wv_f = []
                for i in range(8):
                    wvt = win2.tile([128, 256], F32, tag=f"wvf{i}", name=f"wvf{i}")
                    nc.sync.dma_start(wvt[:], wvf[ts(i, 128), :])
                    wv_f.append(wvt)
                s2_t = rconsts.tile([128, D_K], F32, tag="s2", name="s2")
                nc.sync.dma_start(s2_t[:], s2[:])
                cos_t = rconsts.tile([128, S], F32, tag="cos", name="cos")
                nc.sync.dma_start(cos_t[:], costab[:])
                sin_t = rconsts.tile([128, S], F32, tag="sin", name="sin")
                nc.sync.dma_start(sin_t[:], sintab[:])

                with ExitStack() as ph3a:
                    pvp = ph3a.enter_context(
                        tc.tile_pool(name="pvp", bufs=1, space="PSUM"))
                    pv = [pvp.tile([128, 256], F32, tag=f"pv{p}", name=f"pv{p}")
                          for p in range(8)]
                    for i in range(8):
                        xk_f = xin2.tile([128, S], F32, tag="xkf2", name="xkf2")
                        nc.sync.dma_start(xk_f[:], xkT[ts(i, 128), :])
                        for st in range(8):
                            nc.tensor.matmul(pv[st][:],
                                             xk_f[:, ts(st, 128)], wv_f[i][:],
                                             start=(i == 0), stop=(i == 7))
                    for st in range(8):
                        nc.scalar.copy(v_t[st][:], pv[st][:])

                with ExitStack() as ph3:
                    rotp = ph3.enter_context(
                        tc.tile_pool(name="rotp", bufs=2, space="PSUM"))
                    rtmp = ph3.enter_context(tc.tile_pool(name="rtmp", bufs=2))
                    rfin = ph3.enter_context(tc.tile_pool(name="rfin", bufs=2))

                    # rope: q' = q*cos + (S q)*sin ; split to f32r hi/lo
                    for raw, dst in ((qraw, qr), (kraw, kr)):
                        for t in range(2):
                            rp = rotp.tile([128, S], F32, tag="rot", name="rot")
                            for ch in range(2):
                                nc.tensor.matmul(
                                    rp[0:64, ts(ch, 512)], s2_t[0:64, :],
                                    raw[t][0:64, ts(ch, 512)],
                                    start=True, stop=True, tile_position=(0, 0))
                                nc.tensor.matmul(
                                    rp[64:128, ts(ch, 512)], s2_t[64:128, :],
                                    raw[t][64:128, ts(ch, 512)],
                                    start=True, stop=True, tile_position=(64, 64))
                            tmp = rtmp.tile([128, S], F32, tag="rtmp", name="rtmp")
                            nc.vector.tensor_tensor(tmp[:], rp[:], sin_t[:], ALU.mult)
                            nc.vector.tensor_tensor(dst[t][:], raw[t][:], cos_t[:],
                                                    ALU.mult)
                            nc.vector.tensor_tensor(dst[t][:], dst[t][:], tmp[:],
                                                    ALU.add)

        # consts for later phases
        idn_t = consts.tile([128, 128], F32, tag="idn", name="idn")
        nc.sync.dma_start(idn_t[:], idn[:])
        with tc.tile_pool(name="wstg3", bufs=2) as wstg3:
            wp_h = load_rounded(consts, wstg3, wph, [128, D_MODEL], "wph", 2)
            wp_l = load_rounded(consts, wstg3, wpl, [128, D_MODEL], "wpl", 2)
        mask_t = [mask_pool.tile([128, S], BF16, tag=f"mask{qi}", name=f"mask{qi}")
                  for qi in range(8)]
        for qi in range(8):
            nc.sync.dma_start(mask_t[qi][:], maskn[ts(qi, 128), :])

        # ---------------- phase 4: per head-pair attention
        with ExitStack() as ph:
            natp = ph.enter_context(tc.tile_pool(name="natp", bufs=1, space="PSUM"))
            trp = ph.enter_context(tc.tile_pool(name="trp", bufs=2, space="PSUM"))
            atp = ph.enter_context(tc.tile_pool(name="atp", bufs=1, space="PSUM"))
            mout = ph.enter_context(tc.tile_pool(name="mout", bufs=2))
            pvin = ph.enter_context(tc.tile_pool(name="pvin", bufs=3))
            tsum = ph.enter_context(tc.tile_pool(name="tsum", bufs=2))
            epool = ph.enter_context(tc.tile_pool(name="epool", bufs=2))
            wtp = ph.enter_context(tc.tile_pool(name="wtp", bufs=1))
            rcp = ph.enter_context(tc.tile_pool(name="rcp", bufs=4))

            for t in range(2):
                wta = [wtp.tile([128, 8 * S], F32, tag=f"wta{hl}", name=f"wta{hl}")
                       for hl in range(2)]
                # A) scores -> mask -> DMA -> +prev -> exp -> normalize -> transpose
                for qi in range(8):
                    for hl in range(2):
                        h = 2 * t + hl
                        lo, hi = (0, 64) if hl == 0 else (64, 128)
                        tp = (0, 0) if hl == 0 else (64, 0)
                        ps = natp.tile([128, S], F32, tag="nat", name="nat")
                        for ch in range(2):
                            nc.tensor.matmul(ps[:, ts(ch, 512)],
                                             qr[t][lo:hi, ts(qi, 128)],
                                             kr[t][lo:hi, ts(ch, 512)],
                                             start=True, stop=True,
                                             tile_position=tp)
                        mo = mout.tile([128, S], F32, tag="mo", name="mo")
                        nc.vector.tensor_tensor(mo[:], ps[:], mask_t[qi][:], ALU.add)
                        nc.sync.dma_start(scr[h, ts(qi, 128), :], mo[:])
                        pvt = pvin.tile([128, S], F32, tag="pvt", name="pvt")
                        nc.sync.dma_start(pvt[:], prevh[h, ts(qi, 128), :])
                        tsu = tsum.tile([128, S], F32, tag="tsx", name="tsx")
                        nc.vector.tensor_tensor(tsu[:], mo[:], pvt[:], ALU.add)
                        e = epool.tile([128, S], F32, tag="e", name="e")
                        dn = rcp.tile([128, 1], F32, tag="dn", name="dn")
                        nc.scalar.activation(e[:], tsu[:], AF.Exp,
                                             scale=0.5, accum_out=dn[:])
                        rc = rcp.tile([128, 1], F32, tag="rc", name="rc")
                        nc.vector.tensor_scalar_add(rc[:], dn[:], 1e-30)
                        nc.vector.reciprocal(rc[:], rc[:])
                        nc.vector.tensor_scalar(e[:], e[:], rc[:], None, ALU.mult)
                        if kdbg and t == 0 and hl == 0:
                            nc.sync.dma_start(dbg_e[qi, :, :], e[:])
                        # transpose e's 8 blocks into wta column qi of each kt band
                        wv_kt = wta[hl][:].rearrange("p (kt q) -> p kt q", q=S)
                        for half in range(2):
                            tpp = trp.tile([128, 512], F32, tag="tr", name="tr")
                            for blk in range(4):
                                kt = half * 4 + blk
                                nc.tensor.transpose(
                                    tpp[:, ts(blk, 128)],
                                    e[:, ts(kt, 128)], idn_t[:])
                            nc.scalar.copy(
                                wv_kt[:, half * 4:half * 4 + 4, ts(qi, 128)],
                                tpp[:].rearrange("p (b q) -> p b q", q=128))
                if kdbg and t == 0:
                    nc.sync.dma_start(dbg_w[:], wta[0][:])
                # B) attention x V, accumulated over key tiles (fp32)
                pat = [[atp.tile([64, 512], F32, tag=f"pat{hl}{ch}",
                                 name=f"pat{hl}{ch}") for ch in range(2)]
                       for hl in range(2)]
                for kt in range(8):
                    for ch in range(2):
                        for hl in range(2):
                            coff = 128 * t + 64 * hl
                            nc.tensor.matmul(
                                pat[hl][ch][:],
                                v_t[kt][:, coff:coff + 64],
                                wta[hl][:, kt * S + ch * 512: kt * S + ch * 512 + 512],
                                start=(kt == 0), stop=(kt == 7))
                for ch in range(2):
                    for hl in range(2):
                        lo, hi = (0, 64) if hl == 0 else (64, 128)
                        nc.scalar.copy(ath[t][lo:hi, ts(ch, 512)], pat[hl][ch][:])
                        nc.vector.tensor_tensor(
                            atl[t][lo:hi, ts(ch, 512)], pat[hl][ch][:],
                            ath[t][lo:hi, ts(ch, 512)].bitcast(F32), ALU.subtract)

        if kdbg:
            for st in range(8):
                nc.sync.dma_start(dbg_v[st, :, :], v_t[st][:])
            nc.sync.dma_start(dbg_at[0, :, :], ath[0][:].bitcast(F32))
            nc.sync.dma_start(dbg_at[1, :, :], atl[0][:].bitcast(F32))

        # ---------------- phase 5: output projection (f32r 3-term, partial)
        with ExitStack() as ph:
            pop = ph.enter_context(tc.tile_pool(name="pop", bufs=2, space="PSUM"))
            oout = ph.enter_context(tc.tile_pool(name="oout", bufs=3))
            for et in range(8):
                for ch in range(2):
                    po = pop.tile([128, 512], F32, tag="po", name="po")
                    for t in range(2):
                        _mm3(nc, po[:],
                             wp_h[t][:, ts(et, 128)], wp_l[t][:, ts(et, 128)],
                             ath[t][:, ts(ch, 512)], atl[t][:, ts(ch, 512)],
                             start=(t == 0), stop=(t == 1))
                    oo = oout.tile([128, 512], F32, tag="oo", name="oo")
                    nc.scalar.copy(oo[:], po[:])
                    nc.sync.dma_start(outp[ts(et, 128), ts(ch, 512)], oo[:])

    _split_multi_waits(nc)
    return nc


# ---------------------------------------------------------------- host prep
def _rne11(x):
    """Replicate the device's f32r rounding: RNE to 11 mantissa bits."""
    bits = x.view(np.uint32).astype(np.uint64)
    drop = 12
    half = np.uint64(1 << (drop - 1))
    mask = np.uint64((1 << drop) - 1)
    lsb = (bits >> np.uint64(drop)) & np.uint64(1)
    rem = bits & mask
    add = np.where((rem > half) | ((rem == half) & (lsb == 1)),
                   np.uint64(1 << drop), np.uint64(0))
    out = ((bits + add) >> np.uint64(drop)) << np.uint64(drop)
    return out.astype(np.uint32).view(np.float32)


def _split_hl(x):
    x = np.ascontiguousarray(x, np.float32)
    hi = _rne11(x)
    lo = _rne11((x - hi).astype(np.float32))
    return hi, lo


def _host_tables():
    """Bit-exact replication of reference._rope_tables via jax on CPU."""
    import jax
    import jax.numpy as jnp
    dim = D_K // 2
    cpu = jax.devices("cpu")[0]
    with jax.default_device(cpu):
        theta = jnp.exp(-jnp.arange(dim, dtype=jnp.float32)
                        * (np.log(10000.0) / dim))
        theta = jnp.repeat(theta, 2)
        pos = jnp.arange(1, S + 1, dtype=jnp.float32)[:, None]
        ang = pos * theta
        sin = np.asarray(jnp.sin(ang)).T    # [64, S]
        cos = np.asarray(jnp.cos(ang)).T
    cos128 = np.concatenate([cos, cos], 0).astype(np.float32).copy()
    sin128 = np.concatenate([sin, sin], 0).astype(np.float32).copy()
    s_mat = np.zeros((D_K, D_K), np.float32)
    for i in range(dim):
        s_mat[2 * i + 1, 2 * i] = -1.0
        s_mat[2 * i, 2 * i + 1] = 1.0
    s2 = np.concatenate([s_mat, s_mat], 0).copy()
    return cos128, sin128, s2


def kernel(source_query, source_key_value, source_query_padding_mask,
           source_key_value_padding_mask, prev, Wq, Wk, Wv, Wproj):
    global _PROGRAM, LAST_EXEC_TIME_NS
    _install_ntff_hook()
    if _PROGRAM is None:
        _PROGRAM = _build_program()
    nc = _PROGRAM

    import ml_dtypes

    cos128, sin128, s2 = _host_tables()
    idn = np.eye(128, dtype=np.float32)

    sq = np.asarray(source_query, np.float32)
    skv = np.asarray(source_key_value, np.float32)
    qpad = np.asarray(source_query_padding_mask)
    kpad = np.asarray(source_key_value_padding_mask)
    prev = np.asarray(prev, np.float32)
    Wq = np.asarray(Wq, np.float32)
    Wk = np.asarray(Wk, np.float32)
    Wv = np.asarray(Wv, np.float32)
    Wproj = np.asarray(Wproj, np.float32)
    scale = np.float32(1.0) / np.sqrt(np.float32(D_K))

    tri = np.triu(np.ones((S, S), bool), 1)
    masks = []
    for b in range(B):
        m = np.zeros((S, S), np.float32)
        m[tri] = NEG_INF
        m[:, kpad[b]] = NEG_INF
        m[qpad[b], :] = NEG_INF
        masks.append(m.astype(ml_dtypes.bfloat16))

    xqT = [(sq[b].T * scale).astype(np.float32).copy() for b in range(B)]
    xkT = [skv[b].T.copy() for b in range(B)]

    in_maps = []
    for core in range(N_CORES):
        b = core // 4
        j = core % 4
        sl = slice(256 * j, 256 * (j + 1))
        wq_h, wq_l = _split_hl(Wq[sl, :].T)
        wk_h, wk_l = _split_hl(Wk[sl, :].T)
        wp_h, wp_l = _split_hl(Wproj[:, sl].T)
        in_maps.append(dict(
            xqT=xqT[b], xkT=xkT[b],
            wqh=wq_h, wql=wq_l, wkh=wk_h, wkl=wk_l,
            wvf=np.ascontiguousarray(Wv[sl, :].T), wph=wp_h, wpl=wp_l,
            prevh=np.ascontiguousarray(prev[0, b, 4 * j:4 * j + 4]),
            maskn=masks[b], costab=cos128, sintab=sin128, s2=s2, idn=idn,
        ))

    trace = bool(os.environ.get("KERNEL_TRACE"))
    res = run_bass_kernel_spmd(nc, in_maps, list(range(N_CORES)), trace=trace)
    LAST_EXEC_TIME_NS = res.exec_time_ns
    results = res.results

    scores = np.empty((B, N_HEADS, S, S), np.float32)
    out = np.zeros((B, S, D_MODEL), np.float32)
    for core in range(N_CORES):
        b = core // 4
        j = core % 4
        scores[b, 4 * j:4 * j + 4] = results[core]["scr"]
        out[b] += results[core]["outp"].T
    prev_new = np.concatenate([prev, scores[None]], axis=0)
    return out, prev_new


# revision 19
# speedup vs baseline: 1.0839x; 1.0839x over previous
"""MultiHeadAttention (RoPE, causal+padding masks, score-averaging with prev)
on 8 Trainium2 NeuronCores.

Sharding: batch*heads across cores — core i handles b = i//4 and heads
[4*(i%4) .. 4*(i%4)+3]. Projection weights are sliced per core (tensor
parallel over heads); output-projection partials are summed on the host
during the gather.

Numerics: fp32 data; matmuls for the projections and scores run as
3-term float32r hi/lo splits (f32r = RNE to 11 mantissa bits, full PE
rate; the 3-term split keeps ~2^-22 accuracy, better than a plain fp32
matmul chain). The attention (w @ v) matmul and the exp-tile transposes
stay fp32.
"""
import os
import sys
import types

import numpy as np

try:
    import concourse.bass as bass
except ImportError:
    sys.path.insert(0, "/opt/trn_rl_repo")
    import concourse.bass as bass

import concourse.mybir as mybir
import concourse.tile as tile
from concourse.bass import ts
from concourse.bass_utils import run_bass_kernel_spmd
from contextlib import ExitStack

F32 = mybir.dt.float32
F32R = mybir.dt.float32r
BF16 = mybir.dt.bfloat16
F16 = mybir.dt.float16
AF = mybir.ActivationFunctionType
ALU = mybir.AluOpType

N_HEADS = 16
D_MODEL = 1024
B = 2
S = 1024
D_K = 64
HPC = 4
N_CORES = 8
NEG_INF = float("-inf")

LAST_EXEC_TIME_NS = None


# ---------------------------------------------------------------- wait split
def _split_multi_waits(nc, max_waits=1):
    """This walrus build takes at most one semaphore wait per instruction;
    hoist extras onto NoOps just before it on the same engine stream."""
    n = 0
    for f in nc.m.functions:
        for bb in f.blocks:
            insts = bb.instructions
            if not any(
                i.sync_info and i.sync_info.on_wait
                and len(i.sync_info.on_wait) > max_waits
                for i in insts
            ):
                continue
            new = []
            for inst in insts:
                si = inst.sync_info
                if si is not None and si.on_wait and len(si.on_wait) > max_waits:
                    waits = list(si.on_wait)
                    for w in waits[:-max_waits]:
                        n += 1
                        new.append(mybir.InstNoOp(
                            name=f"{inst.name}-wsplit{n}",
                            engine=inst.engine, ins=[], outs=[],
                            sync_info=mybir.SyncInfo(on_wait=[w], on_update=[]),
                        ))
                    inst.sync_info = mybir.SyncInfo(
                        on_wait=waits[-max_waits:], on_update=list(si.on_update))
                new.append(inst)
            bb.instructions = new
    return n


# ---------------------------------------------------------------- ntff hook
def _install_ntff_hook():
    """Wire up the axon NTFF profile hook missing from this image's antenv
    so trace=True works."""
    if "antenv.axon_hooks" in sys.modules:
        return
    try:
        if "/root/.axon_site" not in sys.path:
            sys.path.insert(0, "/root/.axon_site")
        from trn_agent_boot import trn_boot
        hook = trn_boot._ntff_profile_via_ctypes("/opt/axon/libaxon_pjrt.so")
        mod = types.ModuleType("antenv.axon_hooks")
        holder = [hook]
        mod.get_axon_ntff_profile_hook = lambda: holder[0]
        mod.set_axon_ntff_profile_hook = lambda h: holder.__setitem__(0, h)
        sys.modules["antenv.axon_hooks"] = mod
        import antenv
        antenv.axon_hooks = mod
    except Exception:
        pass


# ---------------------------------------------------------------- program
_PROGRAM = None


def _mm3(nc, out_ap, ah, al, bh, bl, start, stop, tile_position=None):
    """3-term f32r hi/lo matmul accumulation: out += a @ b (a=ah+al, ...)."""
    nc.tensor.matmul(out_ap, ah, bh, start=start, stop=False,
                     tile_position=tile_position)
    nc.tensor.matmul(out_ap, ah, bl, start=False, stop=False,
                     tile_position=tile_position)
    nc.tensor.matmul(out_ap, al, bh, start=False, stop=stop,
                     tile_position=tile_position)


def _build_program():
    nc = bass.Bass()

    xqT = nc.dram_tensor("xqT", [D_MODEL, S], F32, kind="ExternalInput")
    xkT = nc.dram_tensor("xkT", [D_MODEL, S], F32, kind="ExternalInput")
    # host-presplit f32r hi/lo weights (values on the f32r grid, fp32 bits)
    wqh = nc.dram_tensor("wqh", [D_MODEL, 256], F32, kind="ExternalInput")
    wql = nc.dram_tensor("wql", [D_MODEL, 256], F32, kind="ExternalInput")
    wkh = nc.dram_tensor("wkh", [D_MODEL, 256], F32, kind="ExternalInput")
    wkl = nc.dram_tensor("wkl", [D_MODEL, 256], F32, kind="ExternalInput")
    wvf = nc.dram_tensor("wvf", [D_MODEL, 256], F32, kind="ExternalInput")
    wph = nc.dram_tensor("wph", [256, D_MODEL], F32, kind="ExternalInput")
    wpl = nc.dram_tensor("wpl", [256, D_MODEL], F32, kind="ExternalInput")
    prevh = nc.dram_tensor("prevh", [HPC, S, S], F32, kind="ExternalInput")
    maskn = nc.dram_tensor("maskn", [S, S], BF16, kind="ExternalInput")
    costab = nc.dram_tensor("costab", [128, S], F32, kind="ExternalInput")
    sintab = nc.dram_tensor("sintab", [128, S], F32, kind="ExternalInput")
    s2 = nc.dram_tensor("s2", [128, D_K], F32, kind="ExternalInput")
    idn = nc.dram_tensor("idn", [128, 128], F32, kind="ExternalInput")

    scr = nc.dram_tensor("scr", [HPC, S, S], F32, kind="ExternalOutput")
    outp = nc.dram_tensor("outp", [D_MODEL, S], F32, kind="ExternalOutput")
    kdbg = bool(os.environ.get("KDBG"))
    if kdbg:
        dbg_e = nc.dram_tensor("dbg_e", [8, 128, S], F32, kind="ExternalOutput")
        dbg_w = nc.dram_tensor("dbg_w", [128, 8 * S], F32, kind="ExternalOutput")
        dbg_at = nc.dram_tensor("dbg_at", [2, 128, S], F32, kind="ExternalOutput")
        dbg_v = nc.dram_tensor("dbg_v", [8, 128, 256], F32, kind="ExternalOutput")

    with tile.TileContext(nc) as tc, ExitStack() as top:
        consts = top.enter_context(tc.tile_pool(name="consts", bufs=1))
        qk_pool = top.enter_context(tc.tile_pool(name="qk", bufs=1))
        v_pool = top.enter_context(tc.tile_pool(name="vp", bufs=1))
        mask_pool = top.enter_context(tc.tile_pool(name="maskp", bufs=1))
        at_pool = top.enter_context(tc.tile_pool(name="atall", bufs=1))

        # rope'd q/k, f16 hi/lo (partitions: head 2t ch 0-63 | head 2t+1)
        qh16 = [qk_pool.tile([128, S], F16, tag=f"qh{t}", name=f"qh{t}") for t in range(2)]
        ql16 = [qk_pool.tile([128, S], F16, tag=f"ql{t}", name=f"ql{t}") for t in range(2)]
        kh16 = [qk_pool.tile([128, S], F16, tag=f"kh{t}", name=f"kh{t}") for t in range(2)]
        kl16 = [qk_pool.tile([128, S], F16, tag=f"kl{t}", name=f"kl{t}") for t in range(2)]
        v_t = [v_pool.tile([128, 256], F32, tag=f"v{st}", name=f"v{st}") for st in range(8)]
        ath = [at_pool.tile([128, S], F32R, tag=f"ath{t}", name=f"ath{t}") for t in range(2)]
        atl = [at_pool.tile([128, S], F32R, tag=f"atl{t}", name=f"atl{t}") for t in range(2)]

        def load_rounded(pool, stage_pool, dram, shape, tag, n, engine="gpsimd"):
            """DMA fp32 (already on the f32r grid) then identity-cast to f32r
            tiles — the verifier requires a rounding producer for f32r."""
            tiles = []
            for i in range(n):
                st = stage_pool.tile(shape, F32, tag=f"stg_{tag}", name=f"stg_{tag}")
                nc.sync.dma_start(st[:], dram[ts(i, shape[0]), :])
                t_ = pool.tile(shape, F32R, tag=f"{tag}{i}", name=f"{tag}{i}")
                if engine == "gpsimd":
                    nc.gpsimd.tensor_copy(t_[:], st[:])
                elif engine == "vector":
                    nc.vector.tensor_copy(t_[:], st[:])
                else:
                    nc.scalar.copy(t_[:], st[:])
                tiles.append(t_)
            return tiles

        # ---------------- phase 1+2: loads + q/k projections (f32r 3-term)
        with ExitStack() as ph:
            qraw_pool = ph.enter_context(tc.tile_pool(name="qraw", bufs=1))
            phq = ph.enter_context(ExitStack())
            xin = phq.enter_context(tc.tile_pool(name="xin", bufs=2))
            xsp = phq.enter_context(tc.tile_pool(name="xsp", bufs=2))
            win = phq.enter_context(tc.tile_pool(name="win", bufs=1))
            wstg = phq.enter_context(tc.tile_pool(name="wstg", bufs=3))
            pqk = phq.enter_context(tc.tile_pool(name="pqk", bufs=1, space="PSUM"))

            wq_h, wq_l, wk_h, wk_l = [], [], [], []
            for i in range(8):
                for lst, dram, nm in ((wq_h, wqh, "wqh"), (wq_l, wql, "wql"),
                                      (wk_h, wkh, "wkh"), (wk_l, wkl, "wkl")):
                    st_ = wstg.tile([128, 256], F32, tag=f"stg_{nm}", name=f"stg_{nm}")
                    nc.sync.dma_start(st_[:], dram[ts(i, 128), :])
                    t_ = win.tile([128, 256], F32R, tag=f"{nm}{i}", name=f"{nm}{i}")
                    nc.vector.tensor_copy(t_[:], st_[:])
                    lst.append(t_)

            pq = [pqk.tile([128, 512], F32, tag=f"pq{t}{c}", name=f"pq{t}{c}")
                  for t in range(2) for c in range(2)]
            pk = [pqk.tile([128, 512], F32, tag=f"pk{t}{c}", name=f"pk{t}{c}")
                  for t in range(2) for c in range(2)]
            for i in range(8):
                xq_f = xin.tile([128, S], F32, tag="xqf", name="xqf")
                nc.sync.dma_start(xq_f[:], xqT[ts(i, 128), :])
                xk_f = xin.tile([128, S], F32, tag="xkf", name="xkf")
                nc.sync.dma_start(xk_f[:], xkT[ts(i, 128), :])
                xq_h = xsp.tile([128, S], F32R, tag="xqh", name="xqh")
                xq_l = xsp.tile([128, S], F32R, tag="xql", name="xql")
                nc.scalar.copy(xq_h[:], xq_f[:])
                nc.vector.tensor_tensor(xq_l[:], xq_f[:], xq_h[:].bitcast(F32),
                                        ALU.subtract)
                xk_h = xsp.tile([128, S], F32R, tag="xkh", name="xkh")
                xk_l = xsp.tile([128, S], F32R, tag="xkl", name="xkl")
                nc.scalar.copy(xk_h[:], xk_f[:])
                nc.vector.tensor_tensor(xk_l[:], xk_f[:], xk_h[:].bitcast(F32),
                                        ALU.subtract)
                for t in range(2):
                    for c in range(2):
                        _mm3(nc, pq[2 * t + c][:],
                             wq_h[i][:, ts(t, 128)], wq_l[i][:, ts(t, 128)],
                             xq_h[:, ts(c, 512)], xq_l[:, ts(c, 512)],
                             start=(i == 0), stop=(i == 7))
                        _mm3(nc, pk[2 * t + c][:],
                             wk_h[i][:, ts(t, 128)], wk_l[i][:, ts(t, 128)],
                             xk_h[:, ts(c, 512)], xk_l[:, ts(c, 512)],
                             start=(i == 0), stop=(i == 7))

            qraw = [qraw_pool.tile([128, S], F32, tag=f"qq{t}", name=f"qq{t}") for t in range(2)]
            kraw = [qraw_pool.tile([128, S], F32, tag=f"kk{t}", name=f"kk{t}") for t in range(2)]
            for t in range(2):
                for c in range(2):
                    nc.scalar.copy(qraw[t][:, ts(c, 512)], pq[2 * t + c][:])
                    nc.scalar.copy(kraw[t][:, ts(c, 512)], pk[2 * t + c][:])
            phq.close()  # release the q/k psum banks before the v/rope pools

            # ---------------- v projection + rope (same scope so qraw lives)
            with ExitStack() as ph2:
                xin2 = ph2.enter_context(tc.tile_pool(name="xin2", bufs=2))
                xsp2 = ph2.enter_context(tc.tile_pool(name="xsp2", bufs=2))
                win2 = ph2.enter_context(tc.tile_pool(name="win2", bufs=1))
                wstg2 = ph2.enter_context(tc.tile_pool(name="wstg2", bufs=3))
                rconsts = ph2.enter_context(tc.tile_pool(name="rconsts", bufs=1))

                wv_f = []
                for i in range(8):
                    wvt = win2.tile([128, 256], F32, tag=f"wvf{i}", name=f"wvf{i}")
                    nc.sync.dma_start(wvt[:], wvf[ts(i, 128), :])
                    wv_f.append(wvt)
                s2_t = rconsts.tile([128, D_K], F32, tag="s2", name="s2")
                nc.sync.dma_start(s2_t[:], s2[:])
                cos_t = rconsts.tile([128, S], F32, tag="cos", name="cos")
                nc.sync.dma_start(cos_t[:], costab[:])
                sin_t = rconsts.tile([128, S], F32, tag="sin", name="sin")
                nc.sync.dma_start(sin_t[:], sintab[:])

                with ExitStack() as ph3a:
                    pvp = ph3a.enter_context(
                        tc.tile_pool(name="pvp", bufs=1, space="PSUM"))
                    pv = [pvp.tile([128, 256], F32, tag=f"pv{p}", name=f"pv{p}")
                          for p in range(8)]
                    for i in range(8):
                        xk_f = xin2.tile([128, S], F32, tag="xkf2", name="xkf2")
                        nc.sync.dma_start(xk_f[:], xkT[ts(i, 128), :])
                        for st in range(8):
                            nc.tensor.matmul(pv[st][:],
                                             xk_f[:, ts(st, 128)], wv_f[i][:],
                                             start=(i == 0), stop=(i == 7))
                    for st in range(8):
                        nc.scalar.copy(v_t[st][:], pv[st][:])

                with ExitStack() as ph3:
                    rotp = ph3.enter_context(
                        tc.tile_pool(name="rotp", bufs=2, space="PSUM"))
                    rtmp = ph3.enter_context(tc.tile_pool(name="rtmp", bufs=2))
                    rfin = ph3.enter_context(tc.tile_pool(name="rfin", bufs=2))

                    # rope: q' = q*cos + (S q)*sin ; split to f32r hi/lo
                    for raw, dh, dl in ((qraw, qh16, ql16), (kraw, kh16, kl16)):
                        for t in range(2):
                            rp = rotp.tile([128, S], F32, tag="rot", name="rot")
                            for ch in range(2):
                                nc.tensor.matmul(
                                    rp[0:64, ts(ch, 512)], s2_t[0:64, :],
                                    raw[t][0:64, ts(ch, 512)],
                                    start=True, stop=True, tile_position=(0, 0))
                                nc.tensor.matmul(
                                    rp[64:128, ts(ch, 512)], s2_t[64:128, :],
                                    raw[t][64:128, ts(ch, 512)],
                                    start=True, stop=True, tile_position=(64, 64))
                            tmp = rtmp.tile([128, S], F32, tag="rtmp", name="rtmp")
                            nc.vector.tensor_tensor(tmp[:], rp[:], sin_t[:], ALU.mult)
                            fin = rfin.tile([128, S], F32, tag="rfin", name="rfin")
                            nc.vector.tensor_tensor(fin[:], raw[t][:], cos_t[:],
                                                    ALU.mult)
                            nc.vector.tensor_tensor(fin[:], fin[:], tmp[:], ALU.add)
                            nc.scalar.copy(dh[t][:], fin[:])
                            nc.vector.tensor_tensor(dl[t][:], fin[:], dh[t][:],
                                                    ALU.subtract)

        # consts for later phases
        idn_t = consts.tile([128, 128], F32, tag="idn", name="idn")
        nc.sync.dma_start(idn_t[:], idn[:])
        with tc.tile_pool(name="wstg3", bufs=2) as wstg3:
            wp_h = load_rounded(consts, wstg3, wph, [128, D_MODEL], "wph", 2)
            wp_l = load_rounded(consts, wstg3, wpl, [128, D_MODEL], "wpl", 2)
        mask_t = [mask_pool.tile([128, S], BF16, tag=f"mask{qi}", name=f"mask{qi}")
                  for qi in range(8)]
        for qi in range(8):
            nc.sync.dma_start(mask_t[qi][:], maskn[ts(qi, 128), :])

        # ---------------- phase 4: per head-pair attention
        with ExitStack() as ph:
            natp = ph.enter_context(tc.tile_pool(name="natp", bufs=1, space="PSUM"))
            trp = ph.enter_context(tc.tile_pool(name="trp", bufs=2, space="PSUM"))
            atp = ph.enter_context(tc.tile_pool(name="atp", bufs=1, space="PSUM"))
            mout = ph.enter_context(tc.tile_pool(name="mout", bufs=2))
            pvin = ph.enter_context(tc.tile_pool(name="pvin", bufs=3))
            tsum = ph.enter_context(tc.tile_pool(name="tsum", bufs=2))
            epool = ph.enter_context(tc.tile_pool(name="epool", bufs=2))
            wtp = ph.enter_context(tc.tile_pool(name="wtp", bufs=1))
            rcp = ph.enter_context(tc.tile_pool(name="rcp", bufs=4))

            for t in range(2):
                wta = [wtp.tile([128, 8 * S], F32, tag=f"wta{hl}", name=f"wta{hl}")
                       for hl in range(2)]
                # A) scores -> mask -> DMA -> +prev -> exp -> normalize -> transpose
                for qi in range(8):
                    for hl in range(2):
                        h = 2 * t + hl
                        lo, hi = (0, 64) if hl == 0 else (64, 128)
                        tp = (0, 0) if hl == 0 else (64, 0)
                        ps = natp.tile([128, S], F32, tag="nat", name="nat")
                        for ch in range(2):
                            _mm3(nc, ps[:, ts(ch, 512)],
                                 qh16[t][lo:hi, ts(qi, 128)], ql16[t][lo:hi, ts(qi, 128)],
                                 kh16[t][lo:hi, ts(ch, 512)], kl16[t][lo:hi, ts(ch, 512)],
                                 start=True, stop=True, tile_position=tp)
                        mo = mout.tile([128, S], F32, tag="mo", name="mo")
                        nc.vector.tensor_tensor(mo[:], ps[:], mask_t[qi][:], ALU.add)
                        nc.sync.dma_start(scr[h, ts(qi, 128), :], mo[:])
                        pvt = pvin.tile([128, S], F32, tag="pvt", name="pvt")
                        nc.sync.dma_start(pvt[:], prevh[h, ts(qi, 128), :])
                        tsu = tsum.tile([128, S], F32, tag="tsx", name="tsx")
                        nc.vector.tensor_tensor(tsu[:], mo[:], pvt[:], ALU.add)
                        e = epool.tile([128, S], F32, tag="e", name="e")
                        dn = rcp.tile([128, 1], F32, tag="dn", name="dn")
                        nc.scalar.activation(e[:], tsu[:], AF.Exp,
                                             scale=0.5, accum_out=dn[:])
                        rc = rcp.tile([128, 1], F32, tag="rc", name="rc")
                        nc.vector.tensor_scalar_add(rc[:], dn[:], 1e-30)
                        nc.vector.reciprocal(rc[:], rc[:])
                        nc.vector.tensor_scalar(e[:], e[:], rc[:], None, ALU.mult)
                        if kdbg and t == 0 and hl == 0:
                            nc.sync.dma_start(dbg_e[qi, :, :], e[:])
                        # transpose e's 8 blocks into wta column qi of each kt band
                        wv_kt = wta[hl][:].rearrange("p (kt q) -> p kt q", q=S)
                        for half in range(2):
                            tpp = trp.tile([128, 512], F32, tag="tr", name="tr")
                            for blk in range(4):
                                kt = half * 4 + blk
                                nc.tensor.transpose(
                                    tpp[:, ts(blk, 128)],
                                    e[:, ts(kt, 128)], idn_t[:])
                            nc.scalar.copy(
                                wv_kt[:, half * 4:half * 4 + 4, ts(qi, 128)],
                                tpp[:].rearrange("p (b q) -> p b q", q=128))
                if kdbg and t == 0:
                    nc.sync.dma_start(dbg_w[:], wta[0][:])
                # B) attention x V, accumulated over key tiles (fp32)
                pat = [[atp.tile([64, 512], F32, tag=f"pat{hl}{ch}",
                                 name=f"pat{hl}{ch}") for ch in range(2)]
                       for hl in range(2)]
                for kt in range(8):
                    for ch in range(2):
                        for hl in range(2):
                            coff = 128 * t + 64 * hl
                            nc.tensor.matmul(
                                pat[hl][ch][:],
                                v_t[kt][:, coff:coff + 64],
                                wta[hl][:, kt * S + ch * 512: kt * S + ch * 512 + 512],
                                start=(kt == 0), stop=(kt == 7))
                for ch in range(2):
                    for hl in range(2):
                        lo, hi = (0, 64) if hl == 0 else (64, 128)
                        nc.scalar.copy(ath[t][lo:hi, ts(ch, 512)], pat[hl][ch][:])
                        nc.vector.tensor_tensor(
                            atl[t][lo:hi, ts(ch, 512)], pat[hl][ch][:],
                            ath[t][lo:hi, ts(ch, 512)].bitcast(F32), ALU.subtract)

        if kdbg:
            for st in range(8):
                nc.sync.dma_start(dbg_v[st, :, :], v_t[st][:])
            nc.sync.dma_start(dbg_at[0, :, :], ath[0][:].bitcast(F32))
            nc.sync.dma_start(dbg_at[1, :, :], atl[0][:].bitcast(F32))

        # ---------------- phase 5: output projection (f32r 3-term, partial)
        with ExitStack() as ph:
            pop = ph.enter_context(tc.tile_pool(name="pop", bufs=2, space="PSUM"))
            oout = ph.enter_context(tc.tile_pool(name="oout", bufs=3))
            for et in range(8):
                for ch in range(2):
                    po = pop.tile([128, 512], F32, tag="po", name="po")
                    for t in range(2):
                        _mm3(nc, po[:],
                             wp_h[t][:, ts(et, 128)], wp_l[t][:, ts(et, 128)],
                             ath[t][:, ts(ch, 512)], atl[t][:, ts(ch, 512)],
                             start=(t == 0), stop=(t == 1))
                    oo = oout.tile([128, 512], F32, tag="oo", name="oo")
                    nc.scalar.copy(oo[:], po[:])
                    nc.sync.dma_start(outp[ts(et, 128), ts(ch, 512)], oo[:])

    _split_multi_waits(nc)
    return nc


# ---------------------------------------------------------------- host prep
def _rne11(x):
    """Replicate the device's f32r rounding: RNE to 11 mantissa bits."""
    bits = x.view(np.uint32).astype(np.uint64)
    drop = 12
    half = np.uint64(1 << (drop - 1))
    mask = np.uint64((1 << drop) - 1)
    lsb = (bits >> np.uint64(drop)) & np.uint64(1)
    rem = bits & mask
    add = np.where((rem > half) | ((rem == half) & (lsb == 1)),
                   np.uint64(1 << drop), np.uint64(0))
    out = ((bits + add) >> np.uint64(drop)) << np.uint64(drop)
    return out.astype(np.uint32).view(np.float32)


def _split_hl(x):
    x = np.ascontiguousarray(x, np.float32)
    hi = _rne11(x)
    lo = _rne11((x - hi).astype(np.float32))
    return hi, lo


def _host_tables():
    """Bit-exact replication of reference._rope_tables via jax on CPU."""
    import jax
    import jax.numpy as jnp
    dim = D_K // 2
    cpu = jax.devices("cpu")[0]
    with jax.default_device(cpu):
        theta = jnp.exp(-jnp.arange(dim, dtype=jnp.float32)
                        * (np.log(10000.0) / dim))
        theta = jnp.repeat(theta, 2)
        pos = jnp.arange(1, S + 1, dtype=jnp.float32)[:, None]
        ang = pos * theta
        sin = np.asarray(jnp.sin(ang)).T    # [64, S]
        cos = np.asarray(jnp.cos(ang)).T
    cos128 = np.concatenate([cos, cos], 0).astype(np.float32).copy()
    sin128 = np.concatenate([sin, sin], 0).astype(np.float32).copy()
    s_mat = np.zeros((D_K, D_K), np.float32)
    for i in range(dim):
        s_mat[2 * i + 1, 2 * i] = -1.0
        s_mat[2 * i, 2 * i + 1] = 1.0
    s2 = np.concatenate([s_mat, s_mat], 0).copy()
    return cos128, sin128, s2


def kernel(source_query, source_key_value, source_query_padding_mask,
           source_key_value_padding_mask, prev, Wq, Wk, Wv, Wproj):
    global _PROGRAM, LAST_EXEC_TIME_NS
    _install_ntff_hook()
    if _PROGRAM is None:
        _PROGRAM = _build_program()
    nc = _PROGRAM

    import ml_dtypes

    cos128, sin128, s2 = _host_tables()
    idn = np.eye(128, dtype=np.float32)

    sq = np.asarray(source_query, np.float32)
    skv = np.asarray(source_key_value, np.float32)
    qpad = np.asarray(source_query_padding_mask)
    kpad = np.asarray(source_key_value_padding_mask)
    prev = np.asarray(prev, np.float32)
    Wq = np.asarray(Wq, np.float32)
    Wk = np.asarray(Wk, np.float32)
    Wv = np.asarray(Wv, np.float32)
    Wproj = np.asarray(Wproj, np.float32)
    scale = np.float32(1.0) / np.sqrt(np.float32(D_K))

    tri = np.triu(np.ones((S, S), bool), 1)
    masks = []
    for b in range(B):
        m = np.zeros((S, S), np.float32)
        m[tri] = NEG_INF
        m[:, kpad[b]] = NEG_INF
        m[qpad[b], :] = NEG_INF
        masks.append(m.astype(ml_dtypes.bfloat16))

    xqT = [(sq[b].T * scale).astype(np.float32).copy() for b in range(B)]
    xkT = [skv[b].T.copy() for b in range(B)]

    in_maps = []
    for core in range(N_CORES):
        b = core // 4
        j = core % 4
        sl = slice(256 * j, 256 * (j + 1))
        wq_h, wq_l = _split_hl(Wq[sl, :].T)
        wk_h, wk_l = _split_hl(Wk[sl, :].T)
        wp_h, wp_l = _split_hl(Wproj[:, sl].T)
        in_maps.append(dict(
            xqT=xqT[b], xkT=xkT[b],
            wqh=wq_h, wql=wq_l, wkh=wk_h, wkl=wk_l,
            wvf=np.ascontiguousarray(Wv[sl, :].T), wph=wp_h, wpl=wp_l,
            prevh=np.ascontiguousarray(prev[0, b, 4 * j:4 * j + 4]),
            maskn=masks[b], costab=cos128, sintab=sin128, s2=s2, idn=idn,
        ))

    trace = bool(os.environ.get("KERNEL_TRACE"))
    res = run_bass_kernel_spmd(nc, in_maps, list(range(N_CORES)), trace=trace)
    LAST_EXEC_TIME_NS = res.exec_time_ns
    results = res.results

    scores = np.empty((B, N_HEADS, S, S), np.float32)
    out = np.zeros((B, S, D_MODEL), np.float32)
    for core in range(N_CORES):
        b = core // 4
        j = core % 4
        scores[b, 4 * j:4 * j + 4] = results[core]["scr"]
        out[b] += results[core]["outp"].T
    prev_new = np.concatenate([prev, scores[None]], axis=0)
    return out, prev_new


# revision 20
# speedup vs baseline: 1.0863x; 1.0022x over previous
"""MultiHeadAttention (RoPE, causal+padding masks, score-averaging with prev)
on 8 Trainium2 NeuronCores.

Sharding: batch*heads across cores — core i handles b = i//4 and heads
[4*(i%4) .. 4*(i%4)+3]. Projection weights are sliced per core (tensor
parallel over heads); output-projection partials are summed on the host
during the gather.

Numerics: fp32 data; matmuls for the projections and scores run as
3-term float32r hi/lo splits (f32r = RNE to 11 mantissa bits, full PE
rate; the 3-term split keeps ~2^-22 accuracy, better than a plain fp32
matmul chain). The attention (w @ v) matmul and the exp-tile transposes
stay fp32.
"""
import os
import sys
import types

import numpy as np

try:
    import concourse.bass as bass
except ImportError:
    sys.path.insert(0, "/opt/trn_rl_repo")
    import concourse.bass as bass

import concourse.mybir as mybir
import concourse.tile as tile
from concourse.bass import ts
from concourse.bass_utils import run_bass_kernel_spmd
from contextlib import ExitStack

F32 = mybir.dt.float32
F32R = mybir.dt.float32r
BF16 = mybir.dt.bfloat16
F16 = mybir.dt.float16
AF = mybir.ActivationFunctionType
ALU = mybir.AluOpType

N_HEADS = 16
D_MODEL = 1024
B = 2
S = 1024
D_K = 64
HPC = 4
N_CORES = 8
NEG_INF = float("-inf")

LAST_EXEC_TIME_NS = None


# ---------------------------------------------------------------- wait split
def _split_multi_waits(nc, max_waits=1):
    """This walrus build takes at most one semaphore wait per instruction;
    hoist extras onto NoOps just before it on the same engine stream."""
    n = 0
    for f in nc.m.functions:
        for bb in f.blocks:
            insts = bb.instructions
            if not any(
                i.sync_info and i.sync_info.on_wait
                and len(i.sync_info.on_wait) > max_waits
                for i in insts
            ):
                continue
            new = []
            for inst in insts:
                si = inst.sync_info
                if si is not None and si.on_wait and len(si.on_wait) > max_waits:
                    waits = list(si.on_wait)
                    for w in waits[:-max_waits]:
                        n += 1
                        new.append(mybir.InstNoOp(
                            name=f"{inst.name}-wsplit{n}",
                            engine=inst.engine, ins=[], outs=[],
                            sync_info=mybir.SyncInfo(on_wait=[w], on_update=[]),
                        ))
                    inst.sync_info = mybir.SyncInfo(
                        on_wait=waits[-max_waits:], on_update=list(si.on_update))
                new.append(inst)
            bb.instructions = new
    return n


# ---------------------------------------------------------------- ntff hook
def _install_ntff_hook():
    """Wire up the axon NTFF profile hook missing from this image's antenv
    so trace=True works."""
    if "antenv.axon_hooks" in sys.modules:
        return
    try:
        if "/root/.axon_site" not in sys.path:
            sys.path.insert(0, "/root/.axon_site")
        from trn_agent_boot import trn_boot
        hook = trn_boot._ntff_profile_via_ctypes("/opt/axon/libaxon_pjrt.so")
        mod = types.ModuleType("antenv.axon_hooks")
        holder = [hook]
        mod.get_axon_ntff_profile_hook = lambda: holder[0]
        mod.set_axon_ntff_profile_hook = lambda h: holder.__setitem__(0, h)
        sys.modules["antenv.axon_hooks"] = mod
        import antenv
        antenv.axon_hooks = mod
    except Exception:
        pass


# ---------------------------------------------------------------- program
_PROGRAM = None


def _mm3(nc, out_ap, ah, al, bh, bl, start, stop, tile_position=None):
    """3-term f32r hi/lo matmul accumulation: out += a @ b (a=ah+al, ...)."""
    nc.tensor.matmul(out_ap, ah, bh, start=start, stop=False,
                     tile_position=tile_position)
    nc.tensor.matmul(out_ap, ah, bl, start=False, stop=False,
                     tile_position=tile_position)
    nc.tensor.matmul(out_ap, al, bh, start=False, stop=stop,
                     tile_position=tile_position)


def _build_program():
    nc = bass.Bass()

    xqT = nc.dram_tensor("xqT", [D_MODEL, S], F32, kind="ExternalInput")
    xkT = nc.dram_tensor("xkT", [D_MODEL, S], F32, kind="ExternalInput")
    # host-presplit f32r hi/lo weights (values on the f32r grid, fp32 bits)
    wqh = nc.dram_tensor("wqh", [D_MODEL, 256], F32, kind="ExternalInput")
    wql = nc.dram_tensor("wql", [D_MODEL, 256], F32, kind="ExternalInput")
    wkh = nc.dram_tensor("wkh", [D_MODEL, 256], F32, kind="ExternalInput")
    wkl = nc.dram_tensor("wkl", [D_MODEL, 256], F32, kind="ExternalInput")
    wvf = nc.dram_tensor("wvf", [D_MODEL, 256], F32, kind="ExternalInput")
    wph = nc.dram_tensor("wph", [256, D_MODEL], F32, kind="ExternalInput")
    wpl = nc.dram_tensor("wpl", [256, D_MODEL], F32, kind="ExternalInput")
    prevh = nc.dram_tensor("prevh", [HPC, S, S], F32, kind="ExternalInput")
    maskn = nc.dram_tensor("maskn", [S, S], BF16, kind="ExternalInput")
    costab = nc.dram_tensor("costab", [128, S], F32, kind="ExternalInput")
    sintab = nc.dram_tensor("sintab", [128, S], F32, kind="ExternalInput")
    s2 = nc.dram_tensor("s2", [128, D_K], F32, kind="ExternalInput")
    idn = nc.dram_tensor("idn", [128, 128], F32, kind="ExternalInput")

    scr = nc.dram_tensor("scr", [HPC, S, S], F32, kind="ExternalOutput")
    outp = nc.dram_tensor("outp", [D_MODEL, S], F32, kind="ExternalOutput")
    kdbg = bool(os.environ.get("KDBG"))
    if kdbg:
        dbg_e = nc.dram_tensor("dbg_e", [8, 128, S], F32, kind="ExternalOutput")
        dbg_w = nc.dram_tensor("dbg_w", [128, 8 * S], F32, kind="ExternalOutput")
        dbg_at = nc.dram_tensor("dbg_at", [2, 128, S], F32, kind="ExternalOutput")
        dbg_v = nc.dram_tensor("dbg_v", [8, 128, 256], F32, kind="ExternalOutput")

    with tile.TileContext(nc) as tc, ExitStack() as top:
        consts = top.enter_context(tc.tile_pool(name="consts", bufs=1))
        qk_pool = top.enter_context(tc.tile_pool(name="qk", bufs=1))
        v_pool = top.enter_context(tc.tile_pool(name="vp", bufs=1))
        mask_pool = top.enter_context(tc.tile_pool(name="maskp", bufs=1))
        at_pool = top.enter_context(tc.tile_pool(name="atall", bufs=1))

        # rope'd q/k, f16 hi/lo (partitions: head 2t ch 0-63 | head 2t+1)
        qh16 = [qk_pool.tile([128, S], F16, tag=f"qh{t}", name=f"qh{t}") for t in range(2)]
        ql16 = [qk_pool.tile([128, S], F16, tag=f"ql{t}", name=f"ql{t}") for t in range(2)]
        kh16 = [qk_pool.tile([128, S], F16, tag=f"kh{t}", name=f"kh{t}") for t in range(2)]
        kl16 = [qk_pool.tile([128, S], F16, tag=f"kl{t}", name=f"kl{t}") for t in range(2)]
        v_t = [v_pool.tile([128, 256], F32, tag=f"v{st}", name=f"v{st}") for st in range(8)]
        ath = [at_pool.tile([128, S], F32R, tag=f"ath{t}", name=f"ath{t}") for t in range(2)]
        atl = [at_pool.tile([128, S], F32R, tag=f"atl{t}", name=f"atl{t}") for t in range(2)]

        def load_rounded(pool, stage_pool, dram, shape, tag, n, engine="gpsimd"):
            """DMA fp32 (already on the f32r grid) then identity-cast to f32r
            tiles — the verifier requires a rounding producer for f32r."""
            tiles = []
            for i in range(n):
                st = stage_pool.tile(shape, F32, tag=f"stg_{tag}", name=f"stg_{tag}")
                nc.sync.dma_start(st[:], dram[ts(i, shape[0]), :])
                t_ = pool.tile(shape, F32R, tag=f"{tag}{i}", name=f"{tag}{i}")
                if engine == "gpsimd":
                    nc.gpsimd.tensor_copy(t_[:], st[:])
                elif engine == "vector":
                    nc.vector.tensor_copy(t_[:], st[:])
                else:
                    nc.scalar.copy(t_[:], st[:])
                tiles.append(t_)
            return tiles

        # ---------------- phase 1+2: loads + q/k projections (f32r 3-term)
        with ExitStack() as ph:
            qraw_pool = ph.enter_context(tc.tile_pool(name="qraw", bufs=1))
            phq = ph.enter_context(ExitStack())
            xin = phq.enter_context(tc.tile_pool(name="xin", bufs=2))
            xsp = phq.enter_context(tc.tile_pool(name="xsp", bufs=2))
            win = phq.enter_context(tc.tile_pool(name="win", bufs=1))
            wstg = phq.enter_context(tc.tile_pool(name="wstg", bufs=3))
            pqk = phq.enter_context(tc.tile_pool(name="pqk", bufs=1, space="PSUM"))

            wq_h, wq_l, wk_h, wk_l = [], [], [], []
            for i in range(8):
                for lst, dram, nm in ((wq_h, wqh, "wqh"), (wq_l, wql, "wql"),
                                      (wk_h, wkh, "wkh"), (wk_l, wkl, "wkl")):
                    st_ = wstg.tile([128, 256], F32, tag=f"stg_{nm}", name=f"stg_{nm}")
                    nc.sync.dma_start(st_[:], dram[ts(i, 128), :])
                    t_ = win.tile([128, 256], F32R, tag=f"{nm}{i}", name=f"{nm}{i}")
                    nc.vector.tensor_copy(t_[:], st_[:])
                    lst.append(t_)

            pq = [pqk.tile([128, 512], F32, tag=f"pq{t}{c}", name=f"pq{t}{c}")
                  for t in range(2) for c in range(2)]
            pk = [pqk.tile([128, 512], F32, tag=f"pk{t}{c}", name=f"pk{t}{c}")
                  for t in range(2) for c in range(2)]
            for i in range(8):
                xq_f = xin.tile([128, S], F32, tag="xqf", name="xqf")
                nc.sync.dma_start(xq_f[:], xqT[ts(i, 128), :])
                xk_f = xin.tile([128, S], F32, tag="xkf", name="xkf")
                nc.sync.dma_start(xk_f[:], xkT[ts(i, 128), :])
                xq_h = xsp.tile([128, S], F32R, tag="xqh", name="xqh")
                xq_l = xsp.tile([128, S], F32R, tag="xql", name="xql")
                nc.scalar.copy(xq_h[:], xq_f[:])
                nc.vector.tensor_tensor(xq_l[:], xq_f[:], xq_h[:].bitcast(F32),
                                        ALU.subtract)
                xk_h = xsp.tile([128, S], F32R, tag="xkh", name="xkh")
                xk_l = xsp.tile([128, S], F32R, tag="xkl", name="xkl")
                nc.scalar.copy(xk_h[:], xk_f[:])
                nc.vector.tensor_tensor(xk_l[:], xk_f[:], xk_h[:].bitcast(F32),
                                        ALU.subtract)
                for t in range(2):
                    for c in range(2):
                        _mm3(nc, pq[2 * t + c][:],
                             wq_h[i][:, ts(t, 128)], wq_l[i][:, ts(t, 128)],
                             xq_h[:, ts(c, 512)], xq_l[:, ts(c, 512)],
                             start=(i == 0), stop=(i == 7))
                        _mm3(nc, pk[2 * t + c][:],
                             wk_h[i][:, ts(t, 128)], wk_l[i][:, ts(t, 128)],
                             xk_h[:, ts(c, 512)], xk_l[:, ts(c, 512)],
                             start=(i == 0), stop=(i == 7))

            qraw = [qraw_pool.tile([128, S], F32, tag=f"qq{t}", name=f"qq{t}") for t in range(2)]
            kraw = [qraw_pool.tile([128, S], F32, tag=f"kk{t}", name=f"kk{t}") for t in range(2)]
            for t in range(2):
                for c in range(2):
                    nc.scalar.copy(qraw[t][:, ts(c, 512)], pq[2 * t + c][:])
                    nc.scalar.copy(kraw[t][:, ts(c, 512)], pk[2 * t + c][:])
            phq.close()  # release the q/k psum banks before the v/rope pools

            # ---------------- v projection + rope (same scope so qraw lives)
            with ExitStack() as ph2:
                xin2 = ph2.enter_context(tc.tile_pool(name="xin2", bufs=2))
                xsp2 = ph2.enter_context(tc.tile_pool(name="xsp2", bufs=2))
                win2 = ph2.enter_context(tc.tile_pool(name="win2", bufs=1))
                wstg2 = ph2.enter_context(tc.tile_pool(name="wstg2", bufs=3))
                rconsts = ph2.enter_context(tc.tile_pool(name="rconsts", bufs=1))

                wv_f = []
                for i in range(8):
                    wvt = win2.tile([128, 256], F32, tag=f"wvf{i}", name=f"wvf{i}")
                    nc.sync.dma_start(wvt[:], wvf[ts(i, 128), :])
                    wv_f.append(wvt)
                s2_t = rconsts.tile([128, D_K], F32, tag="s2", name="s2")
                nc.sync.dma_start(s2_t[:], s2[:])
                cos_t = rconsts.tile([128, S], F32, tag="cos", name="cos")
                nc.sync.dma_start(cos_t[:], costab[:])
                sin_t = rconsts.tile([128, S], F32, tag="sin", name="sin")
                nc.sync.dma_start(sin_t[:], sintab[:])

                with ExitStack() as ph3a:
                    pvp = ph3a.enter_context(
                        tc.tile_pool(name="pvp", bufs=1, space="PSUM"))
                    pv = [pvp.tile([128, 256], F32, tag=f"pv{p}", name=f"pv{p}")
                          for p in range(8)]
                    for i in range(8):
                        xk_f = xin2.tile([128, S], F32, tag="xkf2", name="xkf2")
                        nc.sync.dma_start(xk_f[:], xkT[ts(i, 128), :])
                        for st in range(8):
                            nc.tensor.matmul(pv[st][:],
                                             xk_f[:, ts(st, 128)], wv_f[i][:],
                                             start=(i == 0), stop=(i == 7))
                    for st in range(8):
                        nc.scalar.copy(v_t[st][:], pv[st][:])

                with ExitStack() as ph3:
                    rotp = ph3.enter_context(
                        tc.tile_pool(name="rotp", bufs=2, space="PSUM"))
                    rtmp = ph3.enter_context(tc.tile_pool(name="rtmp", bufs=2))
                    rfin = ph3.enter_context(tc.tile_pool(name="rfin", bufs=2))

                    # rope: q' = q*cos + (S q)*sin ; split to f32r hi/lo
                    for raw, dh, dl in ((qraw, qh16, ql16), (kraw, kh16, kl16)):
                        for t in range(2):
                            rp = rotp.tile([128, S], F32, tag="rot", name="rot")
                            for ch in range(2):
                                nc.tensor.matmul(
                                    rp[0:64, ts(ch, 512)], s2_t[0:64, :],
                                    raw[t][0:64, ts(ch, 512)],
                                    start=True, stop=True, tile_position=(0, 0))
                                nc.tensor.matmul(
                                    rp[64:128, ts(ch, 512)], s2_t[64:128, :],
                                    raw[t][64:128, ts(ch, 512)],
                                    start=True, stop=True, tile_position=(64, 64))
                            tmp = rtmp.tile([128, S], F32, tag="rtmp", name="rtmp")
                            nc.vector.tensor_tensor(tmp[:], rp[:], sin_t[:], ALU.mult)
                            fin = rfin.tile([128, S], F32, tag="rfin", name="rfin")
                            nc.vector.tensor_tensor(fin[:], raw[t][:], cos_t[:],
                                                    ALU.mult)
                            nc.vector.tensor_tensor(fin[:], fin[:], tmp[:], ALU.add)
                            nc.scalar.copy(dh[t][:], fin[:])
                            nc.vector.tensor_tensor(dl[t][:], fin[:], dh[t][:],
                                                    ALU.subtract)

        # consts for later phases
        idn_t = consts.tile([128, 128], F32, tag="idn", name="idn")
        nc.sync.dma_start(idn_t[:], idn[:])
        with tc.tile_pool(name="wstg3", bufs=2) as wstg3:
            wp_h = load_rounded(consts, wstg3, wph, [128, D_MODEL], "wph", 2)
            wp_l = load_rounded(consts, wstg3, wpl, [128, D_MODEL], "wpl", 2)
        mask_t = [mask_pool.tile([128, S], BF16, tag=f"mask{qi}", name=f"mask{qi}")
                  for qi in range(8)]
        for qi in range(8):
            nc.sync.dma_start(mask_t[qi][:], maskn[ts(qi, 128), :])

        # ---------------- phase 4: per head-pair attention
        with ExitStack() as ph:
            natp = ph.enter_context(tc.tile_pool(name="natp", bufs=1, space="PSUM"))
            trp = ph.enter_context(tc.tile_pool(name="trp", bufs=2, space="PSUM"))
            atp = ph.enter_context(tc.tile_pool(name="atp", bufs=1, space="PSUM"))
            mout = ph.enter_context(tc.tile_pool(name="mout", bufs=3))
            pvin = ph.enter_context(tc.tile_pool(name="pvin", bufs=4))
            tsum = ph.enter_context(tc.tile_pool(name="tsum", bufs=3))
            epool = ph.enter_context(tc.tile_pool(name="epool", bufs=3))
            wtp = ph.enter_context(tc.tile_pool(name="wtp", bufs=1))
            rcp = ph.enter_context(tc.tile_pool(name="rcp", bufs=4))

            for t in range(2):
                wta = [wtp.tile([128, 8 * S], F32, tag=f"wta{hl}", name=f"wta{hl}")
                       for hl in range(2)]
                # A) scores -> mask -> DMA -> +prev -> exp -> normalize -> transpose
                for qi in range(8):
                    for hl in range(2):
                        h = 2 * t + hl
                        lo, hi = (0, 64) if hl == 0 else (64, 128)
                        tp = (0, 0) if hl == 0 else (64, 0)
                        ps = natp.tile([128, S], F32, tag="nat", name="nat")
                        for ch in range(2):
                            _mm3(nc, ps[:, ts(ch, 512)],
                                 qh16[t][lo:hi, ts(qi, 128)], ql16[t][lo:hi, ts(qi, 128)],
                                 kh16[t][lo:hi, ts(ch, 512)], kl16[t][lo:hi, ts(ch, 512)],
                                 start=True, stop=True, tile_position=tp)
                        mo = mout.tile([128, S], F32, tag="mo", name="mo")
                        nc.vector.tensor_tensor(mo[:], ps[:], mask_t[qi][:], ALU.add)
                        nc.sync.dma_start(scr[h, ts(qi, 128), :], mo[:])
                        pvt = pvin.tile([128, S], F32, tag="pvt", name="pvt")
                        nc.sync.dma_start(pvt[:], prevh[h, ts(qi, 128), :])
                        tsu = tsum.tile([128, S], F32, tag="tsx", name="tsx")
                        nc.vector.tensor_tensor(tsu[:], mo[:], pvt[:], ALU.add)
                        e = epool.tile([128, S], F32, tag="e", name="e")
                        dn = rcp.tile([128, 1], F32, tag="dn", name="dn")
                        nc.scalar.activation(e[:], tsu[:], AF.Exp,
                                             scale=0.5, accum_out=dn[:])
                        rc = rcp.tile([128, 1], F32, tag="rc", name="rc")
                        nc.vector.tensor_scalar_add(rc[:], dn[:], 1e-30)
                        nc.vector.reciprocal(rc[:], rc[:])
                        nc.vector.tensor_scalar(e[:], e[:], rc[:], None, ALU.mult)
                        if kdbg and t == 0 and hl == 0:
                            nc.sync.dma_start(dbg_e[qi, :, :], e[:])
                        # transpose e's 8 blocks into wta column qi of each kt band
                        wv_kt = wta[hl][:].rearrange("p (kt q) -> p kt q", q=S)
                        for half in range(2):
                            tpp = trp.tile([128, 512], F32, tag="tr", name="tr")
                            for blk in range(4):
                                kt = half * 4 + blk
                                nc.tensor.transpose(
                                    tpp[:, ts(blk, 128)],
                                    e[:, ts(kt, 128)], idn_t[:])
                            nc.scalar.copy(
                                wv_kt[:, half * 4:half * 4 + 4, ts(qi, 128)],
                                tpp[:].rearrange("p (b q) -> p b q", q=128))
                if kdbg and t == 0:
                    nc.sync.dma_start(dbg_w[:], wta[0][:])
                # B) attention x V, accumulated over key tiles (fp32)
                pat = [[atp.tile([64, 512], F32, tag=f"pat{hl}{ch}",
                                 name=f"pat{hl}{ch}") for ch in range(2)]
                       for hl in range(2)]
                for kt in range(8):
                    for ch in range(2):
                        for hl in range(2):
                            coff = 128 * t + 64 * hl
                            nc.tensor.matmul(
                                pat[hl][ch][:],
                                v_t[kt][:, coff:coff + 64],
                                wta[hl][:, kt * S + ch * 512: kt * S + ch * 512 + 512],
                                start=(kt == 0), stop=(kt == 7))
                for ch in range(2):
                    for hl in range(2):
                        lo, hi = (0, 64) if hl == 0 else (64, 128)
                        nc.scalar.copy(ath[t][lo:hi, ts(ch, 512)], pat[hl][ch][:])
                        nc.vector.tensor_tensor(
                            atl[t][lo:hi, ts(ch, 512)], pat[hl][ch][:],
                            ath[t][lo:hi, ts(ch, 512)].bitcast(F32), ALU.subtract)

        if kdbg:
            for st in range(8):
                nc.sync.dma_start(dbg_v[st, :, :], v_t[st][:])
            nc.sync.dma_start(dbg_at[0, :, :], ath[0][:].bitcast(F32))
            nc.sync.dma_start(dbg_at[1, :, :], atl[0][:].bitcast(F32))

        # ---------------- phase 5: output projection (f32r 3-term, partial)
        with ExitStack() as ph:
            pop = ph.enter_context(tc.tile_pool(name="pop", bufs=2, space="PSUM"))
            oout = ph.enter_context(tc.tile_pool(name="oout", bufs=3))
            for et in range(8):
                for ch in range(2):
                    po = pop.tile([128, 512], F32, tag="po", name="po")
                    for t in range(2):
                        _mm3(nc, po[:],
                             wp_h[t][:, ts(et, 128)], wp_l[t][:, ts(et, 128)],
                             ath[t][:, ts(ch, 512)], atl[t][:, ts(ch, 512)],
                             start=(t == 0), stop=(t == 1))
                    oo = oout.tile([128, 512], F32, tag="oo", name="oo")
                    nc.scalar.copy(oo[:], po[:])
                    nc.sync.dma_start(outp[ts(et, 128), ts(ch, 512)], oo[:])

    _split_multi_waits(nc)
    return nc


# ---------------------------------------------------------------- host prep
def _rne11(x):
    """Replicate the device's f32r rounding: RNE to 11 mantissa bits."""
    bits = x.view(np.uint32).astype(np.uint64)
    drop = 12
    half = np.uint64(1 << (drop - 1))
    mask = np.uint64((1 << drop) - 1)
    lsb = (bits >> np.uint64(drop)) & np.uint64(1)
    rem = bits & mask
    add = np.where((rem > half) | ((rem == half) & (lsb == 1)),
                   np.uint64(1 << drop), np.uint64(0))
    out = ((bits + add) >> np.uint64(drop)) << np.uint64(drop)
    return out.astype(np.uint32).view(np.float32)


def _split_hl(x):
    x = np.ascontiguousarray(x, np.float32)
    hi = _rne11(x)
    lo = _rne11((x - hi).astype(np.float32))
    return hi, lo


def _host_tables():
    """Bit-exact replication of reference._rope_tables via jax on CPU."""
    import jax
    import jax.numpy as jnp
    dim = D_K // 2
    cpu = jax.devices("cpu")[0]
    with jax.default_device(cpu):
        theta = jnp.exp(-jnp.arange(dim, dtype=jnp.float32)
                        * (np.log(10000.0) / dim))
        theta = jnp.repeat(theta, 2)
        pos = jnp.arange(1, S + 1, dtype=jnp.float32)[:, None]
        ang = pos * theta
        sin = np.asarray(jnp.sin(ang)).T    # [64, S]
        cos = np.asarray(jnp.cos(ang)).T
    cos128 = np.concatenate([cos, cos], 0).astype(np.float32).copy()
    sin128 = np.concatenate([sin, sin], 0).astype(np.float32).copy()
    s_mat = np.zeros((D_K, D_K), np.float32)
    for i in range(dim):
        s_mat[2 * i + 1, 2 * i] = -1.0
        s_mat[2 * i, 2 * i + 1] = 1.0
    s2 = np.concatenate([s_mat, s_mat], 0).copy()
    return cos128, sin128, s2


def kernel(source_query, source_key_value, source_query_padding_mask,
           source_key_value_padding_mask, prev, Wq, Wk, Wv, Wproj):
    global _PROGRAM, LAST_EXEC_TIME_NS
    _install_ntff_hook()
    if _PROGRAM is None:
        _PROGRAM = _build_program()
    nc = _PROGRAM

    import ml_dtypes

    cos128, sin128, s2 = _host_tables()
    idn = np.eye(128, dtype=np.float32)

    sq = np.asarray(source_query, np.float32)
    skv = np.asarray(source_key_value, np.float32)
    qpad = np.asarray(source_query_padding_mask)
    kpad = np.asarray(source_key_value_padding_mask)
    prev = np.asarray(prev, np.float32)
    Wq = np.asarray(Wq, np.float32)
    Wk = np.asarray(Wk, np.float32)
    Wv = np.asarray(Wv, np.float32)
    Wproj = np.asarray(Wproj, np.float32)
    scale = np.float32(1.0) / np.sqrt(np.float32(D_K))

    tri = np.triu(np.ones((S, S), bool), 1)
    masks = []
    for b in range(B):
        m = np.zeros((S, S), np.float32)
        m[tri] = NEG_INF
        m[:, kpad[b]] = NEG_INF
        m[qpad[b], :] = NEG_INF
        masks.append(m.astype(ml_dtypes.bfloat16))

    xqT = [(sq[b].T * scale).astype(np.float32).copy() for b in range(B)]
    xkT = [skv[b].T.copy() for b in range(B)]

    in_maps = []
    for core in range(N_CORES):
        b = core // 4
        j = core % 4
        sl = slice(256 * j, 256 * (j + 1))
        wq_h, wq_l = _split_hl(Wq[sl, :].T)
        wk_h, wk_l = _split_hl(Wk[sl, :].T)
        wp_h, wp_l = _split_hl(Wproj[:, sl].T)
        in_maps.append(dict(
            xqT=xqT[b], xkT=xkT[b],
            wqh=wq_h, wql=wq_l, wkh=wk_h, wkl=wk_l,
            wvf=np.ascontiguousarray(Wv[sl, :].T), wph=wp_h, wpl=wp_l,
            prevh=np.ascontiguousarray(prev[0, b, 4 * j:4 * j + 4]),
            maskn=masks[b], costab=cos128, sintab=sin128, s2=s2, idn=idn,
        ))

    trace = bool(os.environ.get("KERNEL_TRACE"))
    res = run_bass_kernel_spmd(nc, in_maps, list(range(N_CORES)), trace=trace)
    LAST_EXEC_TIME_NS = res.exec_time_ns
    results = res.results

    scores = np.empty((B, N_HEADS, S, S), np.float32)
    out = np.zeros((B, S, D_MODEL), np.float32)
    for core in range(N_CORES):
        b = core // 4
        j = core % 4
        scores[b, 4 * j:4 * j + 4] = results[core]["scr"]
        out[b] += results[core]["outp"].T
    prev_new = np.concatenate([prev, scores[None]], axis=0)
    return out, prev_new


# revision 21
# speedup vs baseline: 1.1505x; 1.0591x over previous
"""MultiHeadAttention (RoPE, causal+padding masks, score-averaging with prev)
on 8 Trainium2 NeuronCores.

Sharding: batch*heads across cores — core i handles b = i//4 and heads
[4*(i%4) .. 4*(i%4)+3]. Projection weights are sliced per core (tensor
parallel over heads); output-projection partials are summed on the host
during the gather.

Numerics: fp32 data; matmuls for the projections and scores run as
3-term float32r hi/lo splits (f32r = RNE to 11 mantissa bits, full PE
rate; the 3-term split keeps ~2^-22 accuracy, better than a plain fp32
matmul chain). The attention (w @ v) matmul and the exp-tile transposes
stay fp32.
"""
import os
import sys
import types

import numpy as np

try:
    import concourse.bass as bass
except ImportError:
    sys.path.insert(0, "/opt/trn_rl_repo")
    import concourse.bass as bass

import concourse.mybir as mybir
import concourse.tile as tile
from concourse.bass import ts
from concourse.bass_utils import run_bass_kernel_spmd
from contextlib import ExitStack

F32 = mybir.dt.float32
F32R = mybir.dt.float32r
BF16 = mybir.dt.bfloat16
F16 = mybir.dt.float16
AF = mybir.ActivationFunctionType
ALU = mybir.AluOpType

N_HEADS = 16
D_MODEL = 1024
B = 2
S = 1024
D_K = 64
HPC = 4
N_CORES = 8
NEG_INF = float("-inf")

LAST_EXEC_TIME_NS = None


# ---------------------------------------------------------------- wait split
def _split_multi_waits(nc, max_waits=1):
    """This walrus build takes at most one semaphore wait per instruction;
    hoist extras onto NoOps just before it on the same engine stream."""
    n = 0
    for f in nc.m.functions:
        for bb in f.blocks:
            insts = bb.instructions
            if not any(
                i.sync_info and i.sync_info.on_wait
                and len(i.sync_info.on_wait) > max_waits
                for i in insts
            ):
                continue
            new = []
            for inst in insts:
                si = inst.sync_info
                if si is not None and si.on_wait and len(si.on_wait) > max_waits:
                    waits = list(si.on_wait)
                    for w in waits[:-max_waits]:
                        n += 1
                        new.append(mybir.InstNoOp(
                            name=f"{inst.name}-wsplit{n}",
                            engine=inst.engine, ins=[], outs=[],
                            sync_info=mybir.SyncInfo(on_wait=[w], on_update=[]),
                        ))
                    inst.sync_info = mybir.SyncInfo(
                        on_wait=waits[-max_waits:], on_update=list(si.on_update))
                new.append(inst)
            bb.instructions = new
    return n


# ---------------------------------------------------------------- ntff hook
def _install_ntff_hook():
    """Wire up the axon NTFF profile hook missing from this image's antenv
    so trace=True works."""
    if "antenv.axon_hooks" in sys.modules:
        return
    try:
        if "/root/.axon_site" not in sys.path:
            sys.path.insert(0, "/root/.axon_site")
        from trn_agent_boot import trn_boot
        hook = trn_boot._ntff_profile_via_ctypes("/opt/axon/libaxon_pjrt.so")
        mod = types.ModuleType("antenv.axon_hooks")
        holder = [hook]
        mod.get_axon_ntff_profile_hook = lambda: holder[0]
        mod.set_axon_ntff_profile_hook = lambda h: holder.__setitem__(0, h)
        sys.modules["antenv.axon_hooks"] = mod
        import antenv
        antenv.axon_hooks = mod
    except Exception:
        pass


# ---------------------------------------------------------------- program
_PROGRAM = None


def _mm3(nc, out_ap, ah, al, bh, bl, start, stop, tile_position=None):
    """3-term f32r hi/lo matmul accumulation: out += a @ b (a=ah+al, ...)."""
    nc.tensor.matmul(out_ap, ah, bh, start=start, stop=False,
                     tile_position=tile_position)
    nc.tensor.matmul(out_ap, ah, bl, start=False, stop=False,
                     tile_position=tile_position)
    nc.tensor.matmul(out_ap, al, bh, start=False, stop=stop,
                     tile_position=tile_position)


def _build_program():
    nc = bass.Bass()

    xqT = nc.dram_tensor("xqT", [D_MODEL, S], F32, kind="ExternalInput")
    xkT = nc.dram_tensor("xkT", [D_MODEL, S], F32, kind="ExternalInput")
    # host-presplit f32r hi/lo weights (values on the f32r grid, fp32 bits)
    wqh = nc.dram_tensor("wqh", [D_MODEL, 256], F32, kind="ExternalInput")
    wql = nc.dram_tensor("wql", [D_MODEL, 256], F32, kind="ExternalInput")
    wkh = nc.dram_tensor("wkh", [D_MODEL, 256], F32, kind="ExternalInput")
    wkl = nc.dram_tensor("wkl", [D_MODEL, 256], F32, kind="ExternalInput")
    wvf = nc.dram_tensor("wvf", [D_MODEL, 256], F32, kind="ExternalInput")
    wph = nc.dram_tensor("wph", [256, D_MODEL], F32, kind="ExternalInput")
    wpl = nc.dram_tensor("wpl", [256, D_MODEL], F32, kind="ExternalInput")
    prevh = nc.dram_tensor("prevh", [HPC, S, S], F32, kind="ExternalInput")
    maskn = nc.dram_tensor("maskn", [S, S], BF16, kind="ExternalInput")
    costab = nc.dram_tensor("costab", [128, S], F32, kind="ExternalInput")
    sintab = nc.dram_tensor("sintab", [128, S], F32, kind="ExternalInput")
    s2 = nc.dram_tensor("s2", [128, D_K], F32, kind="ExternalInput")
    idn = nc.dram_tensor("idn", [128, 128], F32, kind="ExternalInput")

    scr = nc.dram_tensor("scr", [HPC, S, S], F32, kind="ExternalOutput")
    outp = nc.dram_tensor("outp", [D_MODEL, S], F32, kind="ExternalOutput")
    kdbg = bool(os.environ.get("KDBG"))
    if kdbg:
        dbg_e = nc.dram_tensor("dbg_e", [8, 128, S], F32, kind="ExternalOutput")
        dbg_w = nc.dram_tensor("dbg_w", [128, 8 * S], F32, kind="ExternalOutput")
        dbg_at = nc.dram_tensor("dbg_at", [2, 128, S], F32, kind="ExternalOutput")
        dbg_v = nc.dram_tensor("dbg_v", [8, 128, 256], F32, kind="ExternalOutput")

    with tile.TileContext(nc) as tc, ExitStack() as top:
        consts = top.enter_context(tc.tile_pool(name="consts", bufs=1))
        qk_pool = top.enter_context(tc.tile_pool(name="qk", bufs=1))
        v_pool = top.enter_context(tc.tile_pool(name="vp", bufs=1))
        mask_pool = top.enter_context(tc.tile_pool(name="maskp", bufs=1))
        at_pool = top.enter_context(tc.tile_pool(name="atall", bufs=1))

        # rope'd q/k, f16 hi/lo (partitions: head 2t ch 0-63 | head 2t+1)
        qh16 = [qk_pool.tile([128, S], F16, tag=f"qh{t}", name=f"qh{t}") for t in range(2)]
        ql16 = [qk_pool.tile([128, S], F16, tag=f"ql{t}", name=f"ql{t}") for t in range(2)]
        kh16 = [qk_pool.tile([128, S], F16, tag=f"kh{t}", name=f"kh{t}") for t in range(2)]
        kl16 = [qk_pool.tile([128, S], F16, tag=f"kl{t}", name=f"kl{t}") for t in range(2)]
        v_t = [v_pool.tile([128, 256], F32, tag=f"v{st}", name=f"v{st}") for st in range(8)]
        ath = [at_pool.tile([128, S], F32R, tag=f"ath{t}", name=f"ath{t}") for t in range(2)]
        atl = [at_pool.tile([128, S], F32R, tag=f"atl{t}", name=f"atl{t}") for t in range(2)]

        def load_rounded(pool, stage_pool, dram, shape, tag, n, engine="gpsimd"):
            """DMA fp32 (already on the f32r grid) then identity-cast to f32r
            tiles — the verifier requires a rounding producer for f32r."""
            tiles = []
            for i in range(n):
                st = stage_pool.tile(shape, F32, tag=f"stg_{tag}", name=f"stg_{tag}")
                nc.sync.dma_start(st[:], dram[ts(i, shape[0]), :])
                t_ = pool.tile(shape, F32R, tag=f"{tag}{i}", name=f"{tag}{i}")
                if engine == "gpsimd":
                    nc.gpsimd.tensor_copy(t_[:], st[:])
                elif engine == "vector":
                    nc.vector.tensor_copy(t_[:], st[:])
                else:
                    nc.scalar.copy(t_[:], st[:])
                tiles.append(t_)
            return tiles

        # ---------------- phase 1+2: loads + q/k projections (f32r 3-term)
        with ExitStack() as ph:
            qraw_pool = ph.enter_context(tc.tile_pool(name="qraw", bufs=1))
            phq = ph.enter_context(ExitStack())
            xin = phq.enter_context(tc.tile_pool(name="xin", bufs=2))
            xsp = phq.enter_context(tc.tile_pool(name="xsp", bufs=2))
            win = phq.enter_context(tc.tile_pool(name="win", bufs=1))
            wstg = phq.enter_context(tc.tile_pool(name="wstg", bufs=3))
            pqk = phq.enter_context(tc.tile_pool(name="pqk", bufs=1, space="PSUM"))

            wq_h, wq_l, wk_h, wk_l = [], [], [], []
            for i in range(8):
                for lst, dram, nm in ((wq_h, wqh, "wqh"), (wq_l, wql, "wql"),
                                      (wk_h, wkh, "wkh"), (wk_l, wkl, "wkl")):
                    st_ = wstg.tile([128, 256], F32, tag=f"stg_{nm}", name=f"stg_{nm}")
                    nc.sync.dma_start(st_[:], dram[ts(i, 128), :])
                    t_ = win.tile([128, 256], F32R, tag=f"{nm}{i}", name=f"{nm}{i}")
                    nc.vector.tensor_copy(t_[:], st_[:])
                    lst.append(t_)

            pq = [pqk.tile([128, 512], F32, tag=f"pq{t}{c}", name=f"pq{t}{c}")
                  for t in range(2) for c in range(2)]
            pk = [pqk.tile([128, 512], F32, tag=f"pk{t}{c}", name=f"pk{t}{c}")
                  for t in range(2) for c in range(2)]
            for i in range(8):
                xq_f = xin.tile([128, S], F32, tag="xqf", name="xqf")
                nc.sync.dma_start(xq_f[:], xqT[ts(i, 128), :])
                xk_f = xin.tile([128, S], F32, tag="xkf", name="xkf")
                nc.sync.dma_start(xk_f[:], xkT[ts(i, 128), :])
                xq_h = xsp.tile([128, S], F32R, tag="xqh", name="xqh")
                xq_l = xsp.tile([128, S], F32R, tag="xql", name="xql")
                nc.scalar.copy(xq_h[:], xq_f[:])
                nc.vector.tensor_tensor(xq_l[:], xq_f[:], xq_h[:].bitcast(F32),
                                        ALU.subtract)
                xk_h = xsp.tile([128, S], F32R, tag="xkh", name="xkh")
                xk_l = xsp.tile([128, S], F32R, tag="xkl", name="xkl")
                nc.scalar.copy(xk_h[:], xk_f[:])
                nc.vector.tensor_tensor(xk_l[:], xk_f[:], xk_h[:].bitcast(F32),
                                        ALU.subtract)
                for t in range(2):
                    for c in range(2):
                        _mm3(nc, pq[2 * t + c][:],
                             wq_h[i][:, ts(t, 128)], wq_l[i][:, ts(t, 128)],
                             xq_h[:, ts(c, 512)], xq_l[:, ts(c, 512)],
                             start=(i == 0), stop=(i == 7))
                        _mm3(nc, pk[2 * t + c][:],
                             wk_h[i][:, ts(t, 128)], wk_l[i][:, ts(t, 128)],
                             xk_h[:, ts(c, 512)], xk_l[:, ts(c, 512)],
                             start=(i == 0), stop=(i == 7))

            qraw = [qraw_pool.tile([128, S], F32, tag=f"qq{t}", name=f"qq{t}") for t in range(2)]
            kraw = [qraw_pool.tile([128, S], F32, tag=f"kk{t}", name=f"kk{t}") for t in range(2)]
            for t in range(2):
                for c in range(2):
                    nc.scalar.copy(qraw[t][:, ts(c, 512)], pq[2 * t + c][:])
                    nc.scalar.copy(kraw[t][:, ts(c, 512)], pk[2 * t + c][:])
            phq.close()  # release the q/k psum banks before the v/rope pools

            # ---------------- v projection + rope (same scope so qraw lives)
            with ExitStack() as ph2:
                xin2 = ph2.enter_context(tc.tile_pool(name="xin2", bufs=2))
                xsp2 = ph2.enter_context(tc.tile_pool(name="xsp2", bufs=2))
                win2 = ph2.enter_context(tc.tile_pool(name="win2", bufs=1))
                wstg2 = ph2.enter_context(tc.tile_pool(name="wstg2", bufs=3))
                rconsts = ph2.enter_context(tc.tile_pool(name="rconsts", bufs=1))

                wv_f = []
                for i in range(8):
                    wvt = win2.tile([128, 256], F32, tag=f"wvf{i}", name=f"wvf{i}")
                    nc.sync.dma_start(wvt[:], wvf[ts(i, 128), :])
                    wv_f.append(wvt)
                s2_t = rconsts.tile([128, D_K], F32, tag="s2", name="s2")
                nc.sync.dma_start(s2_t[:], s2[:])
                cos_t = rconsts.tile([128, S], F32, tag="cos", name="cos")
                nc.sync.dma_start(cos_t[:], costab[:])
                sin_t = rconsts.tile([128, S], F32, tag="sin", name="sin")
                nc.sync.dma_start(sin_t[:], sintab[:])

                with ExitStack() as ph3:
                    rotp = ph3.enter_context(
                        tc.tile_pool(name="rotp", bufs=2, space="PSUM"))
                    rtmp = ph3.enter_context(tc.tile_pool(name="rtmp", bufs=2))
                    rfin = ph3.enter_context(tc.tile_pool(name="rfin", bufs=2))

                    # rope: q' = q*cos + (S q)*sin ; split to f32r hi/lo
                    for raw, dh, dl in ((qraw, qh16, ql16), (kraw, kh16, kl16)):
                        for t in range(2):
                            rp = rotp.tile([128, S], F32, tag="rot", name="rot")
                            for ch in range(2):
                                nc.tensor.matmul(
                                    rp[0:64, ts(ch, 512)], s2_t[0:64, :],
                                    raw[t][0:64, ts(ch, 512)],
                                    start=True, stop=True, tile_position=(0, 0))
                                nc.tensor.matmul(
                                    rp[64:128, ts(ch, 512)], s2_t[64:128, :],
                                    raw[t][64:128, ts(ch, 512)],
                                    start=True, stop=True, tile_position=(64, 64))
                            tmp = rtmp.tile([128, S], F32, tag="rtmp", name="rtmp")
                            nc.vector.tensor_tensor(tmp[:], rp[:], sin_t[:], ALU.mult)
                            fin = rfin.tile([128, S], F32, tag="rfin", name="rfin")
                            nc.vector.tensor_tensor(fin[:], raw[t][:], cos_t[:],
                                                    ALU.mult)
                            nc.vector.tensor_tensor(fin[:], fin[:], tmp[:], ALU.add)
                            nc.scalar.copy(dh[t][:], fin[:])
                            nc.vector.tensor_tensor(dl[t][:], fin[:], dh[t][:],
                                                    ALU.subtract)

                with ExitStack() as ph3a:
                    pvp = ph3a.enter_context(
                        tc.tile_pool(name="pvp", bufs=1, space="PSUM"))
                    pv = [pvp.tile([128, 256], F32, tag=f"pv{p}", name=f"pv{p}")
                          for p in range(8)]
                    for i in range(8):
                        xk_f = xin2.tile([128, S], F32, tag="xkf2", name="xkf2")
                        nc.sync.dma_start(xk_f[:], xkT[ts(i, 128), :])
                        for st in range(8):
                            nc.tensor.matmul(pv[st][:],
                                             xk_f[:, ts(st, 128)], wv_f[i][:],
                                             start=(i == 0), stop=(i == 7))
                    for st in range(8):
                        nc.scalar.copy(v_t[st][:], pv[st][:])

        # consts for later phases
        idn_t = consts.tile([128, 128], F32, tag="idn", name="idn")
        nc.sync.dma_start(idn_t[:], idn[:])
        with tc.tile_pool(name="wstg3", bufs=2) as wstg3:
            wp_h = load_rounded(consts, wstg3, wph, [128, D_MODEL], "wph", 2)
            wp_l = load_rounded(consts, wstg3, wpl, [128, D_MODEL], "wpl", 2)
        mask_t = [mask_pool.tile([128, S], BF16, tag=f"mask{qi}", name=f"mask{qi}")
                  for qi in range(8)]
        for qi in range(8):
            nc.sync.dma_start(mask_t[qi][:], maskn[ts(qi, 128), :])

        # ---------------- phase 4: per head-pair attention
        with ExitStack() as ph:
            natp = ph.enter_context(tc.tile_pool(name="natp", bufs=1, space="PSUM"))
            trp = ph.enter_context(tc.tile_pool(name="trp", bufs=2, space="PSUM"))
            atp = ph.enter_context(tc.tile_pool(name="atp", bufs=1, space="PSUM"))
            mout = ph.enter_context(tc.tile_pool(name="mout", bufs=3))
            pvin = ph.enter_context(tc.tile_pool(name="pvin", bufs=4))
            tsum = ph.enter_context(tc.tile_pool(name="tsum", bufs=3))
            epool = ph.enter_context(tc.tile_pool(name="epool", bufs=3))
            wtp = ph.enter_context(tc.tile_pool(name="wtp", bufs=1))
            rcp = ph.enter_context(tc.tile_pool(name="rcp", bufs=4))

            for t in range(2):
                wta = [wtp.tile([128, 8 * S], F32, tag=f"wta{hl}", name=f"wta{hl}")
                       for hl in range(2)]
                # A) scores -> mask -> DMA -> +prev -> exp -> normalize -> transpose
                for qi in range(8):
                    for hl in range(2):
                        h = 2 * t + hl
                        lo, hi = (0, 64) if hl == 0 else (64, 128)
                        tp = (0, 0) if hl == 0 else (64, 0)
                        ps = natp.tile([128, S], F32, tag="nat", name="nat")
                        for ch in range(2):
                            _mm3(nc, ps[:, ts(ch, 512)],
                                 qh16[t][lo:hi, ts(qi, 128)], ql16[t][lo:hi, ts(qi, 128)],
                                 kh16[t][lo:hi, ts(ch, 512)], kl16[t][lo:hi, ts(ch, 512)],
                                 start=True, stop=True, tile_position=tp)
                        mo = mout.tile([128, S], F32, tag="mo", name="mo")
                        nc.vector.tensor_tensor(mo[:], ps[:], mask_t[qi][:], ALU.add)
                        nc.sync.dma_start(scr[h, ts(qi, 128), :], mo[:])
                        pvt = pvin.tile([128, S], F32, tag="pvt", name="pvt")
                        nc.sync.dma_start(pvt[:], prevh[h, ts(qi, 128), :])
                        tsu = tsum.tile([128, S], F32, tag="tsx", name="tsx")
                        nc.vector.tensor_tensor(tsu[:], mo[:], pvt[:], ALU.add)
                        e = epool.tile([128, S], F32, tag="e", name="e")
                        dn = rcp.tile([128, 1], F32, tag="dn", name="dn")
                        nc.scalar.activation(e[:], tsu[:], AF.Exp,
                                             scale=0.5, accum_out=dn[:])
                        rc = rcp.tile([128, 1], F32, tag="rc", name="rc")
                        nc.vector.tensor_scalar_add(rc[:], dn[:], 1e-30)
                        nc.vector.reciprocal(rc[:], rc[:])
                        nc.vector.tensor_scalar(e[:], e[:], rc[:], None, ALU.mult)
                        if kdbg and t == 0 and hl == 0:
                            nc.sync.dma_start(dbg_e[qi, :, :], e[:])
                        # transpose e's 8 blocks into wta column qi of each kt band
                        wv_kt = wta[hl][:].rearrange("p (kt q) -> p kt q", q=S)
                        for half in range(2):
                            tpp = trp.tile([128, 512], F32, tag="tr", name="tr")
                            for blk in range(4):
                                kt = half * 4 + blk
                                nc.tensor.transpose(
                                    tpp[:, ts(blk, 128)],
                                    e[:, ts(kt, 128)], idn_t[:])
                            nc.scalar.copy(
                                wv_kt[:, half * 4:half * 4 + 4, ts(qi, 128)],
                                tpp[:].rearrange("p (b q) -> p b q", q=128))
                if kdbg and t == 0:
                    nc.sync.dma_start(dbg_w[:], wta[0][:])
                # B) attention x V, accumulated over key tiles (fp32)
                pat = [[atp.tile([64, 512], F32, tag=f"pat{hl}{ch}",
                                 name=f"pat{hl}{ch}") for ch in range(2)]
                       for hl in range(2)]
                for kt in range(8):
                    for ch in range(2):
                        for hl in range(2):
                            coff = 128 * t + 64 * hl
                            nc.tensor.matmul(
                                pat[hl][ch][:],
                                v_t[kt][:, coff:coff + 64],
                                wta[hl][:, kt * S + ch * 512: kt * S + ch * 512 + 512],
                                start=(kt == 0), stop=(kt == 7))
                for ch in range(2):
                    for hl in range(2):
                        lo, hi = (0, 64) if hl == 0 else (64, 128)
                        nc.scalar.copy(ath[t][lo:hi, ts(ch, 512)], pat[hl][ch][:])
                        nc.vector.tensor_tensor(
                            atl[t][lo:hi, ts(ch, 512)], pat[hl][ch][:],
                            ath[t][lo:hi, ts(ch, 512)].bitcast(F32), ALU.subtract)

        if kdbg:
            for st in range(8):
                nc.sync.dma_start(dbg_v[st, :, :], v_t[st][:])
            nc.sync.dma_start(dbg_at[0, :, :], ath[0][:].bitcast(F32))
            nc.sync.dma_start(dbg_at[1, :, :], atl[0][:].bitcast(F32))

        # ---------------- phase 5: output projection (f32r 3-term, partial)
        with ExitStack() as ph:
            pop = ph.enter_context(tc.tile_pool(name="pop", bufs=2, space="PSUM"))
            oout = ph.enter_context(tc.tile_pool(name="oout", bufs=3))
            for et in range(8):
                for ch in range(2):
                    po = pop.tile([128, 512], F32, tag="po", name="po")
                    for t in range(2):
                        _mm3(nc, po[:],
                             wp_h[t][:, ts(et, 128)], wp_l[t][:, ts(et, 128)],
                             ath[t][:, ts(ch, 512)], atl[t][:, ts(ch, 512)],
                             start=(t == 0), stop=(t == 1))
                    oo = oout.tile([128, 512], F32, tag="oo", name="oo")
                    nc.scalar.copy(oo[:], po[:])
                    nc.sync.dma_start(outp[ts(et, 128), ts(ch, 512)], oo[:])

    _split_multi_waits(nc)
    return nc


# ---------------------------------------------------------------- host prep
def _rne11(x):
    """Replicate the device's f32r rounding: RNE to 11 mantissa bits."""
    bits = x.view(np.uint32).astype(np.uint64)
    drop = 12
    half = np.uint64(1 << (drop - 1))
    mask = np.uint64((1 << drop) - 1)
    lsb = (bits >> np.uint64(drop)) & np.uint64(1)
    rem = bits & mask
    add = np.where((rem > half) | ((rem == half) & (lsb == 1)),
                   np.uint64(1 << drop), np.uint64(0))
    out = ((bits + add) >> np.uint64(drop)) << np.uint64(drop)
    return out.astype(np.uint32).view(np.float32)


def _split_hl(x):
    x = np.ascontiguousarray(x, np.float32)
    hi = _rne11(x)
    lo = _rne11((x - hi).astype(np.float32))
    return hi, lo


def _host_tables():
    """Bit-exact replication of reference._rope_tables via jax on CPU."""
    import jax
    import jax.numpy as jnp
    dim = D_K // 2
    cpu = jax.devices("cpu")[0]
    with jax.default_device(cpu):
        theta = jnp.exp(-jnp.arange(dim, dtype=jnp.float32)
                        * (np.log(10000.0) / dim))
        theta = jnp.repeat(theta, 2)
        pos = jnp.arange(1, S + 1, dtype=jnp.float32)[:, None]
        ang = pos * theta
        sin = np.asarray(jnp.sin(ang)).T    # [64, S]
        cos = np.asarray(jnp.cos(ang)).T
    cos128 = np.concatenate([cos, cos], 0).astype(np.float32).copy()
    sin128 = np.concatenate([sin, sin], 0).astype(np.float32).copy()
    s_mat = np.zeros((D_K, D_K), np.float32)
    for i in range(dim):
        s_mat[2 * i + 1, 2 * i] = -1.0
        s_mat[2 * i, 2 * i + 1] = 1.0
    s2 = np.concatenate([s_mat, s_mat], 0).copy()
    return cos128, sin128, s2


def kernel(source_query, source_key_value, source_query_padding_mask,
           source_key_value_padding_mask, prev, Wq, Wk, Wv, Wproj):
    global _PROGRAM, LAST_EXEC_TIME_NS
    _install_ntff_hook()
    if _PROGRAM is None:
        _PROGRAM = _build_program()
    nc = _PROGRAM

    import ml_dtypes

    cos128, sin128, s2 = _host_tables()
    idn = np.eye(128, dtype=np.float32)

    sq = np.asarray(source_query, np.float32)
    skv = np.asarray(source_key_value, np.float32)
    qpad = np.asarray(source_query_padding_mask)
    kpad = np.asarray(source_key_value_padding_mask)
    prev = np.asarray(prev, np.float32)
    Wq = np.asarray(Wq, np.float32)
    Wk = np.asarray(Wk, np.float32)
    Wv = np.asarray(Wv, np.float32)
    Wproj = np.asarray(Wproj, np.float32)
    scale = np.float32(1.0) / np.sqrt(np.float32(D_K))

    tri = np.triu(np.ones((S, S), bool), 1)
    masks = []
    for b in range(B):
        m = np.zeros((S, S), np.float32)
        m[tri] = NEG_INF
        m[:, kpad[b]] = NEG_INF
        m[qpad[b], :] = NEG_INF
        masks.append(m.astype(ml_dtypes.bfloat16))

    xqT = [(sq[b].T * scale).astype(np.float32).copy() for b in range(B)]
    xkT = [skv[b].T.copy() for b in range(B)]

    in_maps = []
    for core in range(N_CORES):
        b = core // 4
        j = core % 4
        sl = slice(256 * j, 256 * (j + 1))
        wq_h, wq_l = _split_hl(Wq[sl, :].T)
        wk_h, wk_l = _split_hl(Wk[sl, :].T)
        wp_h, wp_l = _split_hl(Wproj[:, sl].T)
        in_maps.append(dict(
            xqT=xqT[b], xkT=xkT[b],
            wqh=wq_h, wql=wq_l, wkh=wk_h, wkl=wk_l,
            wvf=np.ascontiguousarray(Wv[sl, :].T), wph=wp_h, wpl=wp_l,
            prevh=np.ascontiguousarray(prev[0, b, 4 * j:4 * j + 4]),
            maskn=masks[b], costab=cos128, sintab=sin128, s2=s2, idn=idn,
        ))

    trace = bool(os.environ.get("KERNEL_TRACE"))
    res = run_bass_kernel_spmd(nc, in_maps, list(range(N_CORES)), trace=trace)
    LAST_EXEC_TIME_NS = res.exec_time_ns
    results = res.results

    scores = np.empty((B, N_HEADS, S, S), np.float32)
    out = np.zeros((B, S, D_MODEL), np.float32)
    for core in range(N_CORES):
        b = core // 4
        j = core % 4
        scores[b, 4 * j:4 * j + 4] = results[core]["scr"]
        out[b] += results[core]["outp"].T
    prev_new = np.concatenate([prev, scores[None]], axis=0)
    return out, prev_new
